# revision 17
# baseline (speedup 1.0000x reference)
"""Distributed attention kernel for one TRN2 chip (8 NeuronCores).

Problem: multi-head cross-attention
  B=4, TQ=512, TKV=4096, D=1024, H=8 heads (head_dim=128)

Sharding (data-parallel x tensor-parallel, per the hint):
  core c in 0..7 -> (batch b = c // 2, head-group g = c % 2)
  Each core computes heads [4g, 4g+4) for its batch (Wq/Wk/Wv column
  shards), pair-exchanges normalized U with core (b, 1-g) via AllGather,
  and computes its own 512-column slice of the output projection.

Pipeline structure (the key perf idea vs a phase-separated kernel):
  The attention math for T-chunk tc-1 is interleaved instruction-by-
  instruction with the K/V projection matmuls of T-chunk tc, so the
  ACT-engine exp and DVE mask/denominator work run entirely under the
  PE-bound projection stream.  The softmax denominator is accumulated
  on DVE (acc += p per double-step, bf16) instead of PE ones-matmuls,
  cutting ~27us of PE streaming.  PSUM budget is exactly 8 banks:
  4 U accumulators (whole kernel) + 2 S-tile banks (single-buffered,
  WAR hidden by the interleave) + 2 projection banks (double-buffered).

  Per-core PE work is the FLOP-minimal 13.96 GFLOP = ~178us of bf16
  streaming; everything else hides under it.

Tail: per-head finalize (den ones-matmul -> clamp -> fast-reciprocal ->
  gpsimd broadcast -> scale) pipelines into the attention drain of the
  last chunk.  There is NO on-device collective: each core emits the
  PARTIAL output projection over the full 1024 out-columns using only
  its own 4 heads (same FLOPs as a half-width 8-head projection), and
  the host sums the two partials of each pair during unsharding.  This
  removes the pair AllGather from the critical path entirely -- a
  tail-synchronizing collective costs its transfer time PLUS the full
  core-launch skew (measured 10-30us run-to-run), which no kernel-side
  scheduling can hide.
"""

import sys

if "/opt/trn_rl_repo" not in sys.path:
    sys.path.insert(0, "/opt/trn_rl_repo")

import numpy as np
import ml_dtypes
from contextlib import ExitStack

B, TQ, TKV, D, H = 4, 512, 4096, 1024, 8
HD = D // H            # 128 head dim
NCORES = 8
GH = H // 2            # heads per core = 4
GD = GH * HD           # 512 cols per head-group
P = 128
KC = D // P            # 8 contraction chunks
NTB = TKV // P         # 32 T-blocks
NTC = TKV // 512       # 8 T-chunks
NOB = GD // P          # 4 output blocks per core (own col half)
SCALE = float(1.0 / np.sqrt(HD))

_CACHED_NC = None


def _build_nc():
    from concourse import mybir, bacc
    from concourse.tile import TileContext

    bf = mybir.dt.bfloat16
    f32 = mybir.dt.float32
    AF = mybir.ActivationFunctionType
    OP = mybir.AluOpType

    nc = bacc.Bacc("TRN2", target_bir_lowering=False, debug=False,
                   num_devices=NCORES)

    # Host pre-tiles everything partition-major so DMAs are 128 x multi-KB
    # contiguous descriptors.
    xqT = nc.dram_tensor("xqT", [P, KC, TQ], bf, kind="ExternalInput")
    xkvT = nc.dram_tensor("xkvT", [P, NTC, KC, 512], bf, kind="ExternalInput")
    maskT = nc.dram_tensor("maskT", [P, NTB, TQ], bf, kind="ExternalInput")
    Wq = nc.dram_tensor("Wq", [P, KC, GD], bf, kind="ExternalInput")
    Wk = nc.dram_tensor("Wk", [P, KC, GD], bf, kind="ExternalInput")
    Wv = nc.dram_tensor("Wv", [P, KC, GD], bf, kind="ExternalInput")
    # own 4 heads' rows of Wo, FULL 1024 output columns
    Wo = nc.dram_tensor("Wo", [P, GH, D], bf, kind="ExternalInput")
    bq = nc.dram_tensor("bq", [P, GH], f32, kind="ExternalInput")
    bk = nc.dram_tensor("bk", [P, GH], f32, kind="ExternalInput")
    bv = nc.dram_tensor("bv", [GD], f32, kind="ExternalInput")
    out = nc.dram_tensor("out", [P, 2 * NOB, TQ], bf, kind="ExternalOutput")

    with TileContext(nc) as tc:
        with ExitStack() as ctx:
            persist = ctx.enter_context(tc.tile_pool(name="persist", bufs=1))
            kvchunk = ctx.enter_context(tc.tile_pool(name="kvchunk", bufs=2))
            work = ctx.enter_context(tc.tile_pool(name="work", bufs=3))
            # PSUM: exactly 8 banks.
            upool = ctx.enter_context(
                tc.tile_pool(name="upool", bufs=4, space="PSUM"))   # 4 banks
            spool = ctx.enter_context(
                tc.tile_pool(name="spool", bufs=1, space="PSUM"))   # 2 banks
            projp = ctx.enter_context(
                tc.tile_pool(name="projp", bufs=2, space="PSUM"))   # 2 banks
            dram = ctx.enter_context(
                tc.tile_pool(name="dram", bufs=1, space="DRAM"))

            # ---- startup DMAs on two HWDGE queues (sync + scalar) ------
            # kc-sliced wq/xq first so the Q projection starts ~9us in; the
            # big wk/xkv0 transfers are interleaved after the first 4 slices
            # so chunk-0 projection data lands before the PE finishes Q.
            wq_sb = persist.tile([P, KC, GD], bf)
            xq_sb = persist.tile([P, KC, TQ], bf)
            wk_sb = persist.tile([P, KC, GD], bf)
            wv_sb = persist.tile([P, KC, GD], bf)
            mask_q = [persist.tile([P, 8, TQ], bf, name=f"mask{q}")
                      for q in range(4)]
            kv_tiles = {}
            t0c = kvchunk.tile([P, KC, 512], bf, name="xkv_t", tag="xkv")
            kv_tiles[0] = t0c
            for kc in range(4):
                nc.sync.dma_start(wq_sb[:, kc:kc + 1, :], Wq.ap()[:, kc:kc + 1, :])
                nc.scalar.dma_start(xq_sb[:, kc:kc + 1, :], xqT.ap()[:, kc:kc + 1, :])
            nc.sync.dma_start(wk_sb[:], Wk.ap())
            nc.scalar.dma_start(t0c[:], xkvT.ap()[:, 0, :, :])
            for kc in range(4, KC):
                nc.sync.dma_start(wq_sb[:, kc:kc + 1, :], Wq.ap()[:, kc:kc + 1, :])
                nc.scalar.dma_start(xq_sb[:, kc:kc + 1, :], xqT.ap()[:, kc:kc + 1, :])
            nc.sync.dma_start(wv_sb[:], Wv.ap())
            nc.scalar.dma_start(mask_q[0][:], maskT.ap()[:, 0:8, :])

            bq_sb = persist.tile([P, GH], f32)
            bk_sb = persist.tile([P, GH], f32)
            bv_row = persist.tile([1, GD], f32)
            nc.sync.dma_start(bq_sb[:], bq.ap())
            nc.sync.dma_start(bk_sb[:], bk.ap())
            nc.sync.dma_start(bv_row[:], bv.ap().unsqueeze(0))
            bv_rep = persist.tile([P, GD], f32)
            nc.gpsimd.partition_broadcast(bv_rep[:], bv_row[:])

            ones_bf = persist.tile([P, 1], bf)
            nc.vector.memset(ones_bf[:], 1.0)

            # den accumulators (bf16; positive sums, relative errors wash)
            acc = [persist.tile([P, 2, TQ], bf, name=f"acc{h}") for h in range(GH)]
            for h in range(GH):
                nc.vector.memset(acc[h][:], 0.0)

            # ---- Q^T = Wq_g^T x_q^T (+bq), kc-major over 4 upool banks --
            qt_sb = persist.tile([P, GH, TQ], bf)
            q_ps = [upool.tile([P, TQ], f32, name="q_ps", tag="u")
                    for _ in range(GH)]
            for kc in range(KC):
                for db in range(GH):
                    nc.tensor.matmul(q_ps[db][:],
                                     wq_sb[:, kc, db * P:(db + 1) * P],
                                     xq_sb[:, kc, :],
                                     start=(kc == 0), stop=(kc == KC - 1))
            for db in range(GH):
                nc.scalar.activation(qt_sb[:, db, :], q_ps[db][:],
                                     AF.Identity, bias=bq_sb[:, db:db + 1])

            # ---- persistent SBUF for the streamed phase ----------------
            kt_sb = persist.tile([P, GH, TKV], bf)
            v_sb = persist.tile([P, NTB, GD], bf)
            wo_sb = persist.tile([P, GH, D], bf)
            ut_sb = persist.tile([P, GH, TQ], bf)
            o_sb = persist.tile([P, 2 * NOB, TQ], bf)

            u_ps = [None] * GH

            # attention double-step state machine (lag-2 U behind S)
            s_tiles = {}
            p_tiles = {}

            def emit_S(ds):
                h, jp = divmod(ds, NTB // 2)
                j0 = 2 * jp
                t2 = spool.tile([P, 2, TQ], f32, name="s2_ps", tag="s")
                for k in range(2):
                    j = j0 + k
                    nc.tensor.matmul(t2[:, k, :],
                                     kt_sb[:, h, j * P:(j + 1) * P],
                                     qt_sb[:, h, :], start=True, stop=True)
                s_tiles[ds] = t2

            def emit_exp_mask(ds):
                h, jp = divmod(ds, NTB // 2)
                j0 = 2 * jp
                t2 = s_tiles.pop(ds)
                p_t = work.tile([P, 2, TQ], bf, tag="p_t", bufs=3)
                nc.scalar.activation(p_t[:], t2[:], AF.Exp, scale=SCALE)
                q, r0 = divmod(j0, 8)
                nc.vector.tensor_tensor(p_t[:], p_t[:],
                                        mask_q[q][:, r0:r0 + 2, :], OP.mult)
                nc.vector.tensor_tensor(acc[h][:], acc[h][:], p_t[:], OP.add)
                p_tiles[ds] = p_t

            def emit_U(ds):
                h, jp = divmod(ds, NTB // 2)
                j0 = 2 * jp
                if jp == 0:
                    u_ps[h] = upool.tile([P, TQ], f32, name="u_ps", tag="u")
                p_t = p_tiles.pop(ds)
                for k in range(2):
                    j = j0 + k
                    nc.tensor.matmul(u_ps[h][:],
                                     v_sb[:, j, h * P:(h + 1) * P],
                                     p_t[:, k, :],
                                     start=(j == 0), stop=(j == NTB - 1))

            # per-head finalize: den -> recip -> broadcast -> scale -> send
            def emit_fin(h):
                dps = projp.tile([P, TQ], f32, name="den_ps", tag="proj")
                for k in range(2):
                    nc.tensor.matmul(dps[0:1, :], ones_bf[:], acc[h][:, k, :],
                                     start=(k == 0), stop=(k == 1))
                den_cl = work.tile([1, TQ], f32, tag="den_cl", bufs=2)
                nc.vector.tensor_scalar(den_cl[:], dps[0:1, :], 1e-20, None,
                                        OP.max)
                recip = work.tile([1, TQ], f32, tag="recip", bufs=2)
                nc.vector.reciprocal_approx_fast(out=recip[:], in_=den_cl[:])
                rc = work.tile([P, TQ], f32, tag="rc_rep", bufs=2)
                nc.gpsimd.partition_broadcast(rc[:], recip[:])
                nc.vector.tensor_tensor(ut_sb[:, h, :], u_ps[h][:],
                                        rc[:], OP.mult)

            # ---- main streamed loop: proj(tc) interleaved with attn(tc-1)
            # per chunk: 8 proj groups (K db0-3, V tb0-3) and 8 double-steps
            # of the previous chunk's attention, round-robined so the PE
            # stream is dense and single-buffered S-psum never stalls.
            NDS = GH * NTB // 2   # 64 double-steps total
            # double-step visit order: chunk-major, head-minor
            ds_order = []
            for tcnk in range(NTC):
                for h in range(GH):
                    for pz in range(2):
                        ds_order.append(h * (NTB // 2) + tcnk * 2 + pz)
            s_q = list(ds_order)        # S-emission queue
            em_q = list(ds_order)       # exp/mask queue
            u_q = list(ds_order)        # U queue
            n_s = n_em = n_u = 0

            def pump(ns, nem, nu):
                # exp first: the next S matmul recycles the single-buffered
                # S psum slot, so its WAR must see the exp reader emitted.
                nonlocal n_s, n_em, n_u
                while n_em < nem and em_q:
                    emit_exp_mask(em_q.pop(0)); n_em += 1
                while n_s < ns and s_q:
                    emit_S(s_q.pop(0)); n_s += 1
                while n_u < nu and u_q:
                    emit_U(u_q.pop(0)); n_u += 1

            for tcnk in range(NTC):
                # stream next chunk + the mask quarter needed one chunk out
                if tcnk + 1 < NTC:
                    t = kvchunk.tile([P, KC, 512], bf, name="xkv_t", tag="xkv")
                    nc.sync.dma_start(t[:], xkvT.ap()[:, tcnk + 1, :, :])
                    kv_tiles[tcnk + 1] = t
                if tcnk in (1, 3, 5):
                    q = (tcnk + 1) // 2
                    nc.sync.dma_start(mask_q[q][:], maskT.ap()[:, 8 * q:8 * q + 8, :])
                if tcnk == 2:
                    nc.scalar.dma_start(wo_sb[:], Wo.ap())
                xkv_t = kv_tiles.pop(tcnk)

                # 8 proj groups interleaved with the attn pipeline of the
                # PREVIOUS chunk (its K/V tiles are fully in SBUF); U lags
                # the S matmuls by 2 double-steps so exp+mask always clear
                # the DVE/ACT queues before the PE reaches the U matmuls.
                base = (tcnk - 1) * 8
                for i in range(8):
                    ps = projp.tile([P, 512], f32, name="proj_ps", tag="proj")
                    if i < 4:
                        db = i
                        for kc in range(KC):
                            nc.tensor.matmul(ps[:],
                                             wk_sb[:, kc, db * P:(db + 1) * P],
                                             xkv_t[:, kc, :],
                                             start=(kc == 0), stop=(kc == KC - 1))
                        nc.scalar.activation(
                            kt_sb[:, db, tcnk * 512:(tcnk + 1) * 512], ps[:],
                            AF.Identity, bias=bk_sb[:, db:db + 1])
                    else:
                        tb = i - 4
                        for kc in range(KC):
                            nc.tensor.matmul(ps[:],
                                             xkv_t[:, kc, tb * P:(tb + 1) * P],
                                             wv_sb[:, kc, :],
                                             start=(kc == 0), stop=(kc == KC - 1))
                        nc.vector.tensor_tensor(
                            v_sb[:, tcnk * 4 + tb, :], ps[:], bv_rep[:], OP.add)
                    # pump the attention pipeline: one ds per group slot
                    tgt = base + i + 1
                    pump(tgt, tgt - 1, tgt - 2)

            # drain: remaining double-steps of chunk 7, finalizing each head
            # as soon as its last U accumulation is emitted (the finalize
            # chain fills the PE idle slots of the ACT-paced drain).
            fin_done = 0
            while s_q or em_q or u_q:
                pump(n_s + 1, n_em + 1, n_u + 1)
                # in ds_order, head h's last U is at position 56 + 2h + 1
                while fin_done < GH and n_u >= 58 + 2 * fin_done:
                    emit_fin(fin_done)
                    fin_done += 1
            while fin_done < GH:
                emit_fin(fin_done)
                fin_done += 1

            # ---- partial out-proj: own 4 heads x FULL 1024 out-cols ----
            # (no collective; the pair partner's 4-head partial is summed
            # on the host).  8 PSUM banks: 4 from upool (U slots freed by
            # the scales), 2 from projp, 2 from the spool tile's halves.
            o_ps = [upool.tile([P, TQ], f32, name="o_ps", tag="u")
                    for _ in range(NOB)]
            o_ps += [projp.tile([P, TQ], f32, name="o_ps2", tag="proj")
                     for _ in range(2)]
            o67 = spool.tile([P, 2, TQ], f32, name="o_ps3", tag="s")
            o_ps += [o67[:, 0, :], o67[:, 1, :]]
            for ob in range(2 * NOB):
                for lh in range(GH):
                    nc.tensor.matmul(o_ps[ob][:],
                                     wo_sb[:, lh, ob * P:(ob + 1) * P],
                                     ut_sb[:, lh, :],
                                     start=(lh == 0), stop=(lh == GH - 1))
                nc.scalar.activation(o_sb[:, ob, :], o_ps[ob][:], AF.Copy)
            nc.sync.dma_start(out.ap()[:, 0:NOB, :], o_sb[:, 0:NOB, :])
            nc.scalar.dma_start(out.ap()[:, NOB:2 * NOB, :],
                                o_sb[:, NOB:2 * NOB, :])

    nc.finalize()
    return nc


def _ptile(a2d, inner):
    """[R, C] row-major -> [P, R//P, C] partition-major, contiguous."""
    r, c = a2d.shape
    return np.ascontiguousarray(
        a2d.reshape(r // P, P, c).transpose(1, 0, 2)).astype(inner)


def _shard_inputs(inputs_q, inputs_kv, attention_mask, Wq, bq, Wk, bk, Wv, bv,
                  Wo, bo):
    bf16 = ml_dtypes.bfloat16
    f32 = np.float32

    xqT = [_ptile(inputs_q[b].T, bf16) for b in range(B)]         # [P,KC,TQ]
    xkvT = [_ptile(inputs_kv[b].T, bf16)                          # [P,NTC,KC,512]
            .reshape(P, KC, NTC, 512).transpose(0, 2, 1, 3).copy()
            for b in range(B)]
    maskT = [_ptile(attention_mask[b].T.astype(np.float32), bf16)  # [P,NTB,TQ]
             for b in range(B)]
    in_maps = []
    for c in range(NCORES):
        b, g = c // 2, c % 2  # pair = (2b, 2b+1)
        sl = slice(g * GD, (g + 1) * GD)
        in_maps.append({
            "xqT": xqT[b],
            "xkvT": xkvT[b],
            "maskT": maskT[b],
            "Wq": _ptile(np.ascontiguousarray(Wq[:, sl]), bf16),
            "Wk": _ptile(np.ascontiguousarray(Wk[:, sl]), bf16),
            "Wv": _ptile(np.ascontiguousarray(Wv[:, sl]), bf16),
            # own 4 heads' ROWS of Wo, full 1024 out-cols: [P, GH, D] bf16
            "Wo": _ptile(np.ascontiguousarray(Wo[sl, :]), bf16),
            "bq": np.ascontiguousarray(
                bq[sl].reshape(GH, P).T).astype(f32),
            "bk": np.ascontiguousarray(
                bk[sl].reshape(GH, P).T).astype(f32),
            "bv": np.ascontiguousarray(bv[sl]).astype(f32),
        })
    return in_maps


def kernel(_trace=False, **inputs):
    global _CACHED_NC
    from concourse import bass_utils

    arrs = {k: np.asarray(v) for k, v in inputs.items()}
    in_maps = _shard_inputs(**arrs)

    if _CACHED_NC is None:
        _CACHED_NC = _build_nc()

    res = bass_utils.run_bass_kernel_spmd(
        _CACHED_NC, in_maps, core_ids=list(range(NCORES)), trace=_trace)

    bo = arrs["bo"].astype(np.float32)
    full = np.empty((B, TQ, D), np.float32)
    for b in range(B):
        # sum the pair's 4-head partials on the host (the "all-reduce")
        acc = np.zeros((TQ, D), np.float32)
        for c in (2 * b, 2 * b + 1):
            o = res.results[c]["out"]  # [P, 8, TQ] bf16, o-col = ob*128+p
            acc += o.transpose(2, 1, 0).reshape(TQ, D).astype(np.float32)
        full[b] = acc + bo
    if _trace:
        return full, res
    return full


# revision 21
# speedup vs baseline: 1.1726x; 1.1726x over previous
"""Distributed attention kernel for one TRN2 chip (8 NeuronCores).

Problem: multi-head cross-attention
  B=4, TQ=512, TKV=4096, D=1024, H=8 heads (head_dim=128)

Sharding (data-parallel x tensor-parallel, per the hint):
  core c in 0..7 -> (batch b = c // 2, head-group g = c % 2)
  Each core computes heads [4g, 4g+4) for its batch (Wq/Wk/Wv column
  shards), pair-exchanges normalized U with core (b, 1-g) via AllGather,
  and computes its own 512-column slice of the output projection.

Pipeline structure (the key perf idea vs a phase-separated kernel):
  The attention math for T-chunk tc-1 is interleaved instruction-by-
  instruction with the K/V projection matmuls of T-chunk tc, so the
  ACT-engine exp and DVE mask/denominator work run entirely under the
  PE-bound projection stream.  The softmax denominator is accumulated
  on DVE (acc += p per double-step, bf16) instead of PE ones-matmuls,
  cutting ~27us of PE streaming.  PSUM budget is exactly 8 banks:
  4 U accumulators (whole kernel) + 2 S-tile banks (single-buffered,
  WAR hidden by the interleave) + 2 projection banks (double-buffered).

  Per-core PE work is the FLOP-minimal 13.96 GFLOP = ~178us of bf16
  streaming; everything else hides under it.

Tail: per-head finalize (den ones-matmul -> clamp -> fast-reciprocal ->
  gpsimd broadcast -> scale) pipelines into the attention drain of the
  last chunk.  There is NO on-device collective: each core emits the
  PARTIAL output projection over the full 1024 out-columns using only
  its own 4 heads (same FLOPs as a half-width 8-head projection), and
  the host sums the two partials of each pair during unsharding.  This
  removes the pair AllGather from the critical path entirely -- a
  tail-synchronizing collective costs its transfer time PLUS the full
  core-launch skew (measured 10-30us run-to-run), which no kernel-side
  scheduling can hide.
"""

import sys

if "/opt/trn_rl_repo" not in sys.path:
    sys.path.insert(0, "/opt/trn_rl_repo")

import numpy as np
import ml_dtypes
from contextlib import ExitStack

B, TQ, TKV, D, H = 4, 512, 4096, 1024, 8
HD = D // H            # 128 head dim
NCORES = 8
GH = H // 2            # heads per core = 4
GD = GH * HD           # 512 cols per head-group
P = 128
KC = D // P            # 8 contraction chunks
NTB = TKV // P         # 32 T-blocks
NTC = TKV // 512       # 8 T-chunks
NOB = GD // P          # 4 output blocks per core (own col half)
SCALE = float(1.0 / np.sqrt(HD))

_CACHED_NC = None


def _build_nc():
    from concourse import mybir, bacc
    from concourse.tile import TileContext

    bf = mybir.dt.bfloat16
    f32 = mybir.dt.float32
    AF = mybir.ActivationFunctionType
    OP = mybir.AluOpType

    nc = bacc.Bacc("TRN2", target_bir_lowering=False, debug=False,
                   num_devices=NCORES)

    # Host pre-tiles everything partition-major so DMAs are 128 x multi-KB
    # contiguous descriptors.
    xqT = nc.dram_tensor("xqT", [P, KC, TQ], bf, kind="ExternalInput")
    xkvT = nc.dram_tensor("xkvT", [P, NTC, KC, 512], bf, kind="ExternalInput")
    maskT = nc.dram_tensor("maskT", [P, NTB, TQ], bf, kind="ExternalInput")
    Wq = nc.dram_tensor("Wq", [P, KC, GD], bf, kind="ExternalInput")
    Wk = nc.dram_tensor("Wk", [P, KC, GD], bf, kind="ExternalInput")
    Wv = nc.dram_tensor("Wv", [P, KC, GD], bf, kind="ExternalInput")
    # own 4 heads' rows of Wo, FULL 1024 output columns
    Wo = nc.dram_tensor("Wo", [P, GH, D], bf, kind="ExternalInput")
    bq = nc.dram_tensor("bq", [P, GH], f32, kind="ExternalInput")
    bk = nc.dram_tensor("bk", [P, GH], f32, kind="ExternalInput")
    bv = nc.dram_tensor("bv", [GD], f32, kind="ExternalInput")
    out = nc.dram_tensor("out", [P, 2 * NOB, TQ], bf, kind="ExternalOutput")

    with TileContext(nc) as tc:
        with ExitStack() as ctx:
            persist = ctx.enter_context(tc.tile_pool(name="persist", bufs=1))
            kvchunk = ctx.enter_context(tc.tile_pool(name="kvchunk", bufs=2))
            work = ctx.enter_context(tc.tile_pool(name="work", bufs=3))
            # PSUM: exactly 8 banks.
            upool = ctx.enter_context(
                tc.tile_pool(name="upool", bufs=4, space="PSUM"))   # 4 banks
            spool = ctx.enter_context(
                tc.tile_pool(name="spool", bufs=1, space="PSUM"))   # 2 banks
            projp = ctx.enter_context(
                tc.tile_pool(name="projp", bufs=2, space="PSUM"))   # 2 banks
            dram = ctx.enter_context(
                tc.tile_pool(name="dram", bufs=1, space="DRAM"))

            # ---- startup DMAs on two HWDGE queues (sync + scalar) ------
            # Tiny bias DMAs go FIRST (a small transfer queued behind a 1MB
            # one completes ~10us late and its dependent copy stalls the
            # PE's psum-recycling chain).  Then kc-sliced wq/xq so the Q
            # projection starts ~9us in, then the 1MB tensors split across
            # the two queues in need-order.
            wq_sb = persist.tile([P, KC, GD], bf)
            xq_sb = persist.tile([P, KC, TQ], bf)
            wk_sb = persist.tile([P, KC, GD], bf)
            wv_sb = persist.tile([P, KC, GD], bf)
            mask_q = [persist.tile([P, 8, TQ], bf, name=f"mask{q}")
                      for q in range(4)]
            bq_sb = persist.tile([P, GH], f32)
            bk_sb = persist.tile([P, GH], f32)
            bv_row = persist.tile([1, GD], f32)
            nc.sync.dma_start(bq_sb[:], bq.ap())
            nc.sync.dma_start(bk_sb[:], bk.ap())
            nc.sync.dma_start(bv_row[:], bv.ap().unsqueeze(0))
            bv_rep = persist.tile([P, GD], f32)
            nc.gpsimd.partition_broadcast(bv_rep[:], bv_row[:])

            kv_tiles = {}
            t0c = kvchunk.tile([P, KC, 512], bf, name="xkv_t", tag="xkv")
            kv_tiles[0] = t0c
            for kc in range(KC):
                nc.sync.dma_start(wq_sb[:, kc:kc + 1, :], Wq.ap()[:, kc:kc + 1, :])
                nc.scalar.dma_start(xq_sb[:, kc:kc + 1, :], xqT.ap()[:, kc:kc + 1, :])
            nc.sync.dma_start(t0c[:], xkvT.ap()[:, 0, :, :])
            nc.scalar.dma_start(wk_sb[:], Wk.ap())
            nc.sync.dma_start(wv_sb[:], Wv.ap())
            nc.scalar.dma_start(mask_q[0][:], maskT.ap()[:, 0:8, :])

            ones_bf = persist.tile([P, 1], bf)
            nc.vector.memset(ones_bf[:], 1.0)

            # den accumulators (bf16; positive sums, relative errors wash)
            acc = [persist.tile([P, 2, TQ], bf, name=f"acc{h}") for h in range(GH)]
            for h in range(GH):
                nc.vector.memset(acc[h][:], 0.0)

            # ---- Q^T = Wq_g^T x_q^T (+bq), kc-major over 4 upool banks --
            qt_sb = persist.tile([P, GH, TQ], bf)
            q_ps = [upool.tile([P, TQ], f32, name="q_ps", tag="u")
                    for _ in range(GH)]
            for kc in range(KC):
                for db in range(GH):
                    nc.tensor.matmul(q_ps[db][:],
                                     wq_sb[:, kc, db * P:(db + 1) * P],
                                     xq_sb[:, kc, :],
                                     start=(kc == 0), stop=(kc == KC - 1))
            for db in range(GH):
                nc.scalar.activation(qt_sb[:, db, :], q_ps[db][:],
                                     AF.Identity, bias=bq_sb[:, db:db + 1])

            # ---- persistent SBUF for the streamed phase ----------------
            kt_sb = persist.tile([P, GH, TKV], bf)
            v_sb = persist.tile([P, NTB, GD], bf)
            wo_sb = persist.tile([P, GH, D], bf)
            ut_sb = persist.tile([P, GH, TQ], bf)
            o_sb = persist.tile([P, 2 * NOB, TQ], bf)

            u_ps = [None] * GH

            # attention double-step state machine (lag-2 U behind S)
            s_tiles = {}
            p_tiles = {}

            def emit_S(ds):
                h, jp = divmod(ds, NTB // 2)
                j0 = 2 * jp
                t2 = spool.tile([P, 2, TQ], f32, name="s2_ps", tag="s")
                for k in range(2):
                    j = j0 + k
                    nc.tensor.matmul(t2[:, k, :],
                                     kt_sb[:, h, j * P:(j + 1) * P],
                                     qt_sb[:, h, :], start=True, stop=True)
                s_tiles[ds] = t2

            def emit_exp_mask(ds):
                h, jp = divmod(ds, NTB // 2)
                j0 = 2 * jp
                t2 = s_tiles.pop(ds)
                p_t = work.tile([P, 2, TQ], bf, tag="p_t", bufs=3)
                if jp >= 14:
                    # final-chunk drain: split the exp per psum bank so the
                    # next S matmul's WAR on the single S-slot releases
                    # after half an exp instead of a whole one
                    nc.scalar.activation(p_t[:, 0, :], t2[:, 0, :], AF.Exp,
                                         scale=SCALE)
                    nc.scalar.activation(p_t[:, 1, :], t2[:, 1, :], AF.Exp,
                                         scale=SCALE)
                else:
                    nc.scalar.activation(p_t[:], t2[:], AF.Exp, scale=SCALE)
                q, r0 = divmod(j0, 8)
                nc.vector.tensor_tensor(p_t[:], p_t[:],
                                        mask_q[q][:, r0:r0 + 2, :], OP.mult)
                nc.vector.tensor_tensor(acc[h][:], acc[h][:], p_t[:], OP.add)
                p_tiles[ds] = p_t

            def emit_U(ds):
                h, jp = divmod(ds, NTB // 2)
                j0 = 2 * jp
                if jp == 0:
                    u_ps[h] = upool.tile([P, TQ], f32, name="u_ps", tag="u")
                p_t = p_tiles.pop(ds)
                for k in range(2):
                    j = j0 + k
                    nc.tensor.matmul(u_ps[h][:],
                                     v_sb[:, j, h * P:(h + 1) * P],
                                     p_t[:, k, :],
                                     start=(j == 0), stop=(j == NTB - 1))

            # per-head finalize: den -> recip -> broadcast -> scale -> send
            def emit_fin(h):
                dps = projp.tile([P, TQ], f32, name="den_ps", tag="proj")
                for k in range(2):
                    nc.tensor.matmul(dps[0:1, :], ones_bf[:], acc[h][:, k, :],
                                     start=(k == 0), stop=(k == 1))
                den_cl = work.tile([1, TQ], f32, tag="den_cl", bufs=2)
                nc.vector.tensor_scalar(den_cl[:], dps[0:1, :], 1e-20, None,
                                        OP.max)
                recip = work.tile([1, TQ], f32, tag="recip", bufs=2)
                nc.vector.reciprocal_approx_fast(out=recip[:], in_=den_cl[:])
                rc = work.tile([P, TQ], f32, tag="rc_rep", bufs=2)
                nc.gpsimd.partition_broadcast(rc[:], recip[:])
                nc.vector.tensor_tensor(ut_sb[:, h, :], u_ps[h][:],
                                        rc[:], OP.mult)

            # ---- main streamed loop: proj(tc) interleaved with attn(tc-1)
            # per chunk: 8 proj groups (K db0-3, V tb0-3) and 8 double-steps
            # of the previous chunk's attention, round-robined so the PE
            # stream is dense and single-buffered S-psum never stalls.
            NDS = GH * NTB // 2   # 64 double-steps total
            # double-step visit order: chunk-major, head-minor
            ds_order = []
            for tcnk in range(NTC):
                for h in range(GH):
                    for pz in range(2):
                        ds_order.append(h * (NTB // 2) + tcnk * 2 + pz)
            s_q = list(ds_order)        # S-emission queue
            em_q = list(ds_order)       # exp/mask queue
            u_q = list(ds_order)        # U queue
            n_s = n_em = n_u = 0

            def pump(ns, nem, nu):
                # exp first: the next S matmul recycles the single-buffered
                # S psum slot, so its WAR must see the exp reader emitted.
                nonlocal n_s, n_em, n_u
                while n_em < nem and em_q:
                    emit_exp_mask(em_q.pop(0)); n_em += 1
                while n_s < ns and s_q:
                    emit_S(s_q.pop(0)); n_s += 1
                while n_u < nu and u_q:
                    emit_U(u_q.pop(0)); n_u += 1

            for tcnk in range(NTC):
                # stream next chunk + the mask quarter needed one chunk out
                if tcnk + 1 < NTC:
                    t = kvchunk.tile([P, KC, 512], bf, name="xkv_t", tag="xkv")
                    nc.sync.dma_start(t[:], xkvT.ap()[:, tcnk + 1, :, :])
                    kv_tiles[tcnk + 1] = t
                if tcnk in (1, 3, 5):
                    q = (tcnk + 1) // 2
                    nc.sync.dma_start(mask_q[q][:], maskT.ap()[:, 8 * q:8 * q + 8, :])
                if tcnk == 2:
                    nc.scalar.dma_start(wo_sb[:], Wo.ap())
                xkv_t = kv_tiles.pop(tcnk)

                # 8 proj groups interleaved with the attn pipeline of the
                # PREVIOUS chunk (its K/V tiles are fully in SBUF); U lags
                # the S matmuls by 2 double-steps so exp+mask always clear
                # the DVE/ACT queues before the PE reaches the U matmuls.
                base = (tcnk - 1) * 8
                for i in range(8):
                    # chunk 0's V groups borrow the 4 idle upool banks so
                    # the K and V psum-recycle chains start out independent
                    if tcnk == 0 and i >= 4:
                        ps = upool.tile([P, 512], f32, name="v0_ps", tag="u")
                    else:
                        ps = projp.tile([P, 512], f32, name="proj_ps", tag="proj")
                    if i < 4:
                        db = i
                        for kc in range(KC):
                            nc.tensor.matmul(ps[:],
                                             wk_sb[:, kc, db * P:(db + 1) * P],
                                             xkv_t[:, kc, :],
                                             start=(kc == 0), stop=(kc == KC - 1))
                        nc.scalar.activation(
                            kt_sb[:, db, tcnk * 512:(tcnk + 1) * 512], ps[:],
                            AF.Identity, bias=bk_sb[:, db:db + 1])
                    else:
                        tb = i - 4
                        for kc in range(KC):
                            nc.tensor.matmul(ps[:],
                                             xkv_t[:, kc, tb * P:(tb + 1) * P],
                                             wv_sb[:, kc, :],
                                             start=(kc == 0), stop=(kc == KC - 1))
                        nc.vector.tensor_tensor(
                            v_sb[:, tcnk * 4 + tb, :], ps[:], bv_rep[:], OP.add)
                    # pump the attention pipeline: one ds per group slot
                    tgt = base + i + 1
                    pump(tgt, tgt - 1, tgt - 2)

            # drain: remaining double-steps of chunk 7, finalizing each head
            # as soon as its last U accumulation is emitted (the finalize
            # chain fills the PE idle slots of the ACT-paced drain).
            fin_done = 0
            while s_q or em_q or u_q:
                pump(n_s + 1, n_em + 1, n_u + 1)
                # in ds_order, head h's last U is at position 56 + 2h + 1
                while fin_done < GH and n_u >= 58 + 2 * fin_done:
                    emit_fin(fin_done)
                    fin_done += 1
            while fin_done < GH:
                emit_fin(fin_done)
                fin_done += 1

            # ---- partial out-proj: own 4 heads x FULL 1024 out-cols ----
            # (no collective; the pair partner's 4-head partial is summed
            # on the host).  8 PSUM banks: 4 from upool (U slots freed by
            # the scales), 2 from projp, 2 from the spool tile's halves.
            o_ps = [upool.tile([P, TQ], f32, name="o_ps", tag="u")
                    for _ in range(NOB)]
            o_ps += [projp.tile([P, TQ], f32, name="o_ps2", tag="proj")
                     for _ in range(2)]
            o67 = spool.tile([P, 2, TQ], f32, name="o_ps3", tag="s")
            o_ps += [o67[:, 0, :], o67[:, 1, :]]
            for ob in range(2 * NOB):
                for lh in range(GH):
                    nc.tensor.matmul(o_ps[ob][:],
                                     wo_sb[:, lh, ob * P:(ob + 1) * P],
                                     ut_sb[:, lh, :],
                                     start=(lh == 0), stop=(lh == GH - 1))
                # alternate ACT/DVE so the 8 tail copies run ~2-wide
                if ob % 2 == 0:
                    nc.scalar.activation(o_sb[:, ob, :], o_ps[ob][:], AF.Copy)
                else:
                    nc.vector.tensor_scalar(o_sb[:, ob, :], o_ps[ob][:],
                                            0.0, None, OP.add)
            nc.sync.dma_start(out.ap()[:, 0:NOB, :], o_sb[:, 0:NOB, :])
            nc.scalar.dma_start(out.ap()[:, NOB:2 * NOB, :],
                                o_sb[:, NOB:2 * NOB, :])

    nc.finalize()
    return nc


def _ptile(a2d, inner):
    """[R, C] row-major -> [P, R//P, C] partition-major, contiguous."""
    r, c = a2d.shape
    return np.ascontiguousarray(
        a2d.reshape(r // P, P, c).transpose(1, 0, 2)).astype(inner)


def _shard_inputs(inputs_q, inputs_kv, attention_mask, Wq, bq, Wk, bk, Wv, bv,
                  Wo, bo):
    bf16 = ml_dtypes.bfloat16
    f32 = np.float32

    xqT = [_ptile(inputs_q[b].T, bf16) for b in range(B)]         # [P,KC,TQ]
    xkvT = [_ptile(inputs_kv[b].T, bf16)                          # [P,NTC,KC,512]
            .reshape(P, KC, NTC, 512).transpose(0, 2, 1, 3).copy()
            for b in range(B)]
    maskT = [_ptile(attention_mask[b].T.astype(np.float32), bf16)  # [P,NTB,TQ]
             for b in range(B)]
    in_maps = []
    for c in range(NCORES):
        b, g = c // 2, c % 2  # pair = (2b, 2b+1)
        sl = slice(g * GD, (g + 1) * GD)
        in_maps.append({
            "xqT": xqT[b],
            "xkvT": xkvT[b],
            "maskT": maskT[b],
            "Wq": _ptile(np.ascontiguousarray(Wq[:, sl]), bf16),
            "Wk": _ptile(np.ascontiguousarray(Wk[:, sl]), bf16),
            "Wv": _ptile(np.ascontiguousarray(Wv[:, sl]), bf16),
            # own 4 heads' ROWS of Wo, full 1024 out-cols: [P, GH, D] bf16
            "Wo": _ptile(np.ascontiguousarray(Wo[sl, :]), bf16),
            "bq": np.ascontiguousarray(
                bq[sl].reshape(GH, P).T).astype(f32),
            "bk": np.ascontiguousarray(
                bk[sl].reshape(GH, P).T).astype(f32),
            "bv": np.ascontiguousarray(bv[sl]).astype(f32),
        })
    return in_maps


def kernel(_trace=False, **inputs):
    global _CACHED_NC
    from concourse import bass_utils

    arrs = {k: np.asarray(v) for k, v in inputs.items()}
    in_maps = _shard_inputs(**arrs)

    if _CACHED_NC is None:
        _CACHED_NC = _build_nc()

    res = bass_utils.run_bass_kernel_spmd(
        _CACHED_NC, in_maps, core_ids=list(range(NCORES)), trace=_trace)

    bo = arrs["bo"].astype(np.float32)
    full = np.empty((B, TQ, D), np.float32)
    for b in range(B):
        # sum the pair's 4-head partials on the host (the "all-reduce")
        acc = np.zeros((TQ, D), np.float32)
        for c in (2 * b, 2 * b + 1):
            o = res.results[c]["out"]  # [P, 8, TQ] bf16, o-col = ob*128+p
            acc += o.transpose(2, 1, 0).reshape(TQ, D).astype(np.float32)
        full[b] = acc + bo
    if _trace:
        return full, res
    return full


# revision 24
# speedup vs baseline: 1.2137x; 1.0351x over previous
"""Distributed attention kernel for one TRN2 chip (8 NeuronCores).

Problem: multi-head cross-attention
  B=4, TQ=512, TKV=4096, D=1024, H=8 heads (head_dim=128)

Sharding (data-parallel x tensor-parallel, per the hint):
  core c in 0..7 -> (batch b = c // 2, head-group g = c % 2)
  Each core computes heads [4g, 4g+4) for its batch (Wq/Wk/Wv column
  shards), pair-exchanges normalized U with core (b, 1-g) via AllGather,
  and computes its own 512-column slice of the output projection.

Pipeline structure (the key perf idea vs a phase-separated kernel):
  The attention math for T-chunk tc-1 is interleaved instruction-by-
  instruction with the K/V projection matmuls of T-chunk tc, so the
  ACT-engine exp and DVE mask/denominator work run entirely under the
  PE-bound projection stream.  The softmax denominator is accumulated
  on DVE (acc += p per double-step, bf16) instead of PE ones-matmuls,
  cutting ~27us of PE streaming.  PSUM budget is exactly 8 banks:
  4 U accumulators (whole kernel) + 2 S-tile banks (single-buffered,
  WAR hidden by the interleave) + 2 projection banks (double-buffered).

  Per-core PE work is the FLOP-minimal 13.96 GFLOP = ~178us of bf16
  streaming; everything else hides under it.

Tail: per-head finalize (den ones-matmul -> clamp -> fast-reciprocal ->
  gpsimd broadcast -> scale) pipelines into the attention drain of the
  last chunk.  There is NO on-device collective: each core emits the
  PARTIAL output projection over the full 1024 out-columns using only
  its own 4 heads (same FLOPs as a half-width 8-head projection), and
  the host sums the two partials of each pair during unsharding.  This
  removes the pair AllGather from the critical path entirely -- a
  tail-synchronizing collective costs its transfer time PLUS the full
  core-launch skew (measured 10-30us run-to-run), which no kernel-side
  scheduling can hide.
"""

import sys

if "/opt/trn_rl_repo" not in sys.path:
    sys.path.insert(0, "/opt/trn_rl_repo")

import numpy as np
import ml_dtypes
from contextlib import ExitStack

B, TQ, TKV, D, H = 4, 512, 4096, 1024, 8
HD = D // H            # 128 head dim
NCORES = 8
GH = H // 2            # heads per core = 4
GD = GH * HD           # 512 cols per head-group
P = 128
KC = D // P            # 8 contraction chunks
NTB = TKV // P         # 32 T-blocks
NTC = TKV // 512       # 8 T-chunks
NOB = GD // P          # 4 output blocks per core (own col half)
SCALE = float(1.0 / np.sqrt(HD))

_CACHED_NC = None


def _build_nc():
    from concourse import mybir, bacc
    from concourse.tile import TileContext

    bf = mybir.dt.bfloat16
    f32 = mybir.dt.float32
    AF = mybir.ActivationFunctionType
    OP = mybir.AluOpType

    nc = bacc.Bacc("TRN2", target_bir_lowering=False, debug=False,
                   num_devices=NCORES)

    # Host pre-tiles everything partition-major so DMAs are 128 x multi-KB
    # contiguous descriptors.
    xqT = nc.dram_tensor("xqT", [P, KC, TQ], bf, kind="ExternalInput")
    xkvT = nc.dram_tensor("xkvT", [P, NTC, KC, 512], bf, kind="ExternalInput")
    maskT = nc.dram_tensor("maskT", [P, NTB, TQ], bf, kind="ExternalInput")
    Wq = nc.dram_tensor("Wq", [P, KC, GD], bf, kind="ExternalInput")
    Wk = nc.dram_tensor("Wk", [P, KC, GD], bf, kind="ExternalInput")
    Wv = nc.dram_tensor("Wv", [P, KC, GD], bf, kind="ExternalInput")
    # own 4 heads' rows of Wo, FULL 1024 output columns
    Wo = nc.dram_tensor("Wo", [P, GH, D], bf, kind="ExternalInput")
    bq = nc.dram_tensor("bq", [P, GH], f32, kind="ExternalInput")
    bk = nc.dram_tensor("bk", [P, GH], f32, kind="ExternalInput")
    bv = nc.dram_tensor("bv", [GD], f32, kind="ExternalInput")
    out = nc.dram_tensor("out", [P, 2 * NOB, TQ], bf, kind="ExternalOutput")

    with TileContext(nc) as tc:
        with ExitStack() as ctx:
            persist = ctx.enter_context(tc.tile_pool(name="persist", bufs=1))
            kvchunk = ctx.enter_context(tc.tile_pool(name="kvchunk", bufs=2))
            work = ctx.enter_context(tc.tile_pool(name="work", bufs=3))
            # PSUM: exactly 8 banks.
            upool = ctx.enter_context(
                tc.tile_pool(name="upool", bufs=4, space="PSUM"))   # 4 banks
            spool = ctx.enter_context(
                tc.tile_pool(name="spool", bufs=1, space="PSUM"))   # 2 banks
            projp = ctx.enter_context(
                tc.tile_pool(name="projp", bufs=2, space="PSUM"))   # 2 banks
            dram = ctx.enter_context(
                tc.tile_pool(name="dram", bufs=1, space="DRAM"))

            # ---- startup DMAs on two HWDGE queues (sync + scalar) ------
            # Tiny bias DMAs go FIRST (a small transfer queued behind a 1MB
            # one completes ~10us late and its dependent copy stalls the
            # PE's psum-recycling chain).  Then kc-sliced wq/xq so the Q
            # projection starts ~9us in, then the 1MB tensors split across
            # the two queues in need-order.
            wq_sb = persist.tile([P, KC, GD], bf)
            xq_sb = persist.tile([P, KC, TQ], bf)
            wk_sb = persist.tile([P, KC, GD], bf)
            wv_sb = persist.tile([P, KC, GD], bf)
            mask_q = [persist.tile([P, 8, TQ], bf, name=f"mask{q}")
                      for q in range(4)]
            # Each HWDGE queue has 4 DMA rings; the first round of DMAs
            # completes ~4.5us after issue (cold engines) and round 2 waits
            # for ring credits.  Round 1 therefore carries exactly the data
            # the first ~20us of compute needs; biases ride round 2.
            kv_tiles = {}
            t0c = kvchunk.tile([P, KC, 512], bf, name="xkv_t", tag="xkv")
            kv_tiles[0] = t0c
            t1c = kvchunk.tile([P, KC, 512], bf, name="xkv_t", tag="xkv")
            kv_tiles[1] = t1c
            # round 1
            nc.sync.dma_start(wq_sb[:, 0:4, :], Wq.ap()[:, 0:4, :])
            nc.scalar.dma_start(xq_sb[:, 0:4, :], xqT.ap()[:, 0:4, :])
            nc.sync.dma_start(wq_sb[:, 4:8, :], Wq.ap()[:, 4:8, :])
            nc.scalar.dma_start(xq_sb[:, 4:8, :], xqT.ap()[:, 4:8, :])
            nc.sync.dma_start(t0c[:], xkvT.ap()[:, 0, :, :])
            nc.scalar.dma_start(wk_sb[:], Wk.ap())
            nc.sync.dma_start(wv_sb[:], Wv.ap())
            nc.scalar.dma_start(mask_q[0][:], maskT.ap()[:, 0:8, :])
            # round 2
            bq_sb = persist.tile([P, GH], f32)
            bk_sb = persist.tile([P, GH], f32)
            bv_row = persist.tile([1, GD], f32)
            nc.sync.dma_start(bq_sb[:], bq.ap())
            nc.sync.dma_start(bk_sb[:], bk.ap())
            nc.sync.dma_start(bv_row[:], bv.ap().unsqueeze(0))
            nc.sync.dma_start(t1c[:], xkvT.ap()[:, 1, :, :])
            bv_rep = persist.tile([P, GD], f32)
            nc.gpsimd.partition_broadcast(bv_rep[:], bv_row[:])

            ones_bf = persist.tile([P, 1], bf)
            nc.vector.memset(ones_bf[:], 1.0)

            # den accumulators (bf16; positive sums, relative errors wash)
            acc = [persist.tile([P, 2, TQ], bf, name=f"acc{h}") for h in range(GH)]
            for h in range(GH):
                nc.vector.memset(acc[h][:], 0.0)

            # ---- Q^T = Wq_g^T x_q^T (+bq), kc-major over 4 upool banks --
            qt_sb = persist.tile([P, GH, TQ], bf)
            q_ps = [upool.tile([P, TQ], f32, name="q_ps", tag="u")
                    for _ in range(GH)]
            for kc in range(KC):
                for db in range(GH):
                    nc.tensor.matmul(q_ps[db][:],
                                     wq_sb[:, kc, db * P:(db + 1) * P],
                                     xq_sb[:, kc, :],
                                     start=(kc == 0), stop=(kc == KC - 1))
            for db in range(GH):
                nc.scalar.activation(qt_sb[:, db, :], q_ps[db][:],
                                     AF.Identity, bias=bq_sb[:, db:db + 1])

            # ---- persistent SBUF for the streamed phase ----------------
            kt_sb = persist.tile([P, GH, TKV], bf)
            v_sb = persist.tile([P, NTB, GD], bf)
            wo_sb = persist.tile([P, GH, D], bf)
            ut_sb = persist.tile([P, GH, TQ], bf)
            o_sb = persist.tile([P, 2 * NOB, TQ], bf)

            u_ps = [None] * GH

            # attention double-step state machine (lag-2 U behind S)
            s_tiles = {}
            p_tiles = {}

            def emit_S(ds):
                h, jp = divmod(ds, NTB // 2)
                j0 = 2 * jp
                t2 = spool.tile([P, 2, TQ], f32, name="s2_ps", tag="s")
                for k in range(2):
                    j = j0 + k
                    nc.tensor.matmul(t2[:, k, :],
                                     kt_sb[:, h, j * P:(j + 1) * P],
                                     qt_sb[:, h, :], start=True, stop=True)
                s_tiles[ds] = t2

            def emit_exp_mask(ds):
                h, jp = divmod(ds, NTB // 2)
                j0 = 2 * jp
                t2 = s_tiles.pop(ds)
                p_t = work.tile([P, 2, TQ], bf, tag="p_t", bufs=3)
                if jp >= 14:
                    # final-chunk drain: split the exp per psum bank so the
                    # next S matmul's WAR on the single S-slot releases
                    # after half an exp instead of a whole one
                    nc.scalar.activation(p_t[:, 0, :], t2[:, 0, :], AF.Exp,
                                         scale=SCALE)
                    nc.scalar.activation(p_t[:, 1, :], t2[:, 1, :], AF.Exp,
                                         scale=SCALE)
                else:
                    nc.scalar.activation(p_t[:], t2[:], AF.Exp, scale=SCALE)
                q, r0 = divmod(j0, 8)
                nc.vector.tensor_tensor(p_t[:], p_t[:],
                                        mask_q[q][:, r0:r0 + 2, :], OP.mult)
                if jp < 14:
                    nc.vector.tensor_tensor(acc[h][:], acc[h][:], p_t[:],
                                            OP.add)
                else:
                    # final chunk: the den contribution comes from direct
                    # ones-matmuls on the p tiles (PE is idle in the drain,
                    # DVE is the drain's scarce engine)
                    p7_tiles[(h, jp)] = p_t
                p_tiles[ds] = p_t

            def emit_U(ds):
                h, jp = divmod(ds, NTB // 2)
                j0 = 2 * jp
                if jp == 0:
                    u_ps[h] = upool.tile([P, TQ], f32, name="u_ps", tag="u")
                p_t = p_tiles.pop(ds)
                for k in range(2):
                    j = j0 + k
                    nc.tensor.matmul(u_ps[h][:],
                                     v_sb[:, j, h * P:(h + 1) * P],
                                     p_t[:, k, :],
                                     start=(j == 0), stop=(j == NTB - 1))

            # per-head finalize: den -> recip -> broadcast -> scale -> send
            def emit_fin(h):
                dps = projp.tile([P, TQ], f32, name="den_ps", tag="proj")
                for k in range(2):
                    nc.tensor.matmul(dps[0:1, :], ones_bf[:], acc[h][:, k, :],
                                     start=(k == 0), stop=(k == 1))
                den_cl = work.tile([1, TQ], f32, tag="den_cl", bufs=2)
                nc.vector.tensor_scalar(den_cl[:], dps[0:1, :], 1e-20, None,
                                        OP.max)
                recip = work.tile([1, TQ], f32, tag="recip", bufs=2)
                nc.vector.reciprocal_approx_fast(out=recip[:], in_=den_cl[:])
                rc = work.tile([P, TQ], f32, tag="rc_rep", bufs=2)
                nc.gpsimd.partition_broadcast(rc[:], recip[:])
                nc.vector.tensor_tensor(ut_sb[:, h, :], u_ps[h][:],
                                        rc[:], OP.mult)

            # ---- main streamed loop: proj(tc) interleaved with attn(tc-1)
            # per chunk: 8 proj groups (K db0-3, V tb0-3) and 8 double-steps
            # of the previous chunk's attention, round-robined so the PE
            # stream is dense and single-buffered S-psum never stalls.
            NDS = GH * NTB // 2   # 64 double-steps total
            # double-step visit order: chunk-major, head-minor
            ds_order = []
            for tcnk in range(NTC):
                for h in range(GH):
                    for pz in range(2):
                        ds_order.append(h * (NTB // 2) + tcnk * 2 + pz)
            s_q = list(ds_order)        # S-emission queue
            em_q = list(ds_order)       # exp/mask queue
            u_q = list(ds_order)        # U queue
            n_s = n_em = n_u = 0

            def pump(ns, nem, nu):
                # exp first: the next S matmul recycles the single-buffered
                # S psum slot, so its WAR must see the exp reader emitted.
                nonlocal n_s, n_em, n_u
                while n_em < nem and em_q:
                    emit_exp_mask(em_q.pop(0)); n_em += 1
                while n_s < ns and s_q:
                    emit_S(s_q.pop(0)); n_s += 1
                while n_u < nu and u_q:
                    emit_U(u_q.pop(0)); n_u += 1

            for tcnk in range(NTC):
                # stream next chunk + the mask quarter needed one chunk out
                if 1 <= tcnk < NTC - 1:
                    t = kvchunk.tile([P, KC, 512], bf, name="xkv_t", tag="xkv")
                    nc.sync.dma_start(t[:], xkvT.ap()[:, tcnk + 1, :, :])
                    kv_tiles[tcnk + 1] = t
                if tcnk in (1, 3, 5):
                    q = (tcnk + 1) // 2
                    nc.sync.dma_start(mask_q[q][:], maskT.ap()[:, 8 * q:8 * q + 8, :])
                if tcnk == 2:
                    nc.scalar.dma_start(wo_sb[:], Wo.ap())
                xkv_t = kv_tiles.pop(tcnk)

                # 8 proj groups interleaved with the attn pipeline of the
                # PREVIOUS chunk (its K/V tiles are fully in SBUF); U lags
                # the S matmuls by 2 double-steps so exp+mask always clear
                # the DVE/ACT queues before the PE reaches the U matmuls.
                base = (tcnk - 1) * 8
                for i in range(8):
                    # chunk 0's V groups borrow the 4 idle upool banks so
                    # the K and V psum-recycle chains start out independent
                    if tcnk == 0 and i >= 4:
                        ps = upool.tile([P, 512], f32, name="v0_ps", tag="u")
                    else:
                        ps = projp.tile([P, 512], f32, name="proj_ps", tag="proj")
                    if i < 4:
                        db = i
                        for kc in range(KC):
                            nc.tensor.matmul(ps[:],
                                             wk_sb[:, kc, db * P:(db + 1) * P],
                                             xkv_t[:, kc, :],
                                             start=(kc == 0), stop=(kc == KC - 1))
                        nc.scalar.activation(
                            kt_sb[:, db, tcnk * 512:(tcnk + 1) * 512], ps[:],
                            AF.Identity, bias=bk_sb[:, db:db + 1])
                    else:
                        tb = i - 4
                        for kc in range(KC):
                            nc.tensor.matmul(ps[:],
                                             xkv_t[:, kc, tb * P:(tb + 1) * P],
                                             wv_sb[:, kc, :],
                                             start=(kc == 0), stop=(kc == KC - 1))
                        nc.vector.tensor_tensor(
                            v_sb[:, tcnk * 4 + tb, :], ps[:], bv_rep[:], OP.add)
                    # pump the attention pipeline: one ds per group slot
                    tgt = base + i + 1
                    pump(tgt, tgt - 1, tgt - 2)

            # drain: remaining double-steps of chunk 7, finalizing each head
            # as soon as its last U accumulation is emitted (the finalize
            # chain fills the PE idle slots of the ACT-paced drain).
            fin_done = 0
            while s_q or em_q or u_q:
                pump(n_s + 1, n_em + 1, n_u + 1)
                # in ds_order, head h's last U is at position 56 + 2h + 1
                while fin_done < GH and n_u >= 58 + 2 * fin_done:
                    emit_fin(fin_done)
                    fin_done += 1
            while fin_done < GH:
                emit_fin(fin_done)
                fin_done += 1

            # ---- partial out-proj: own 4 heads x FULL 1024 out-cols ----
            # (no collective; the pair partner's 4-head partial is summed
            # on the host).  8 PSUM banks: 4 from upool (U slots freed by
            # the scales), 2 from projp, 2 from the spool tile's halves.
            o_ps = [upool.tile([P, TQ], f32, name="o_ps", tag="u")
                    for _ in range(NOB)]
            o_ps += [projp.tile([P, TQ], f32, name="o_ps2", tag="proj")
                     for _ in range(2)]
            o67 = spool.tile([P, 2, TQ], f32, name="o_ps3", tag="s")
            o_ps += [o67[:, 0, :], o67[:, 1, :]]
            for ob in range(2 * NOB):
                for lh in range(GH):
                    nc.tensor.matmul(o_ps[ob][:],
                                     wo_sb[:, lh, ob * P:(ob + 1) * P],
                                     ut_sb[:, lh, :],
                                     start=(lh == 0), stop=(lh == GH - 1))
                # alternate ACT/DVE so the 8 tail copies run ~2-wide
                if ob % 2 == 0:
                    nc.scalar.activation(o_sb[:, ob, :], o_ps[ob][:], AF.Copy)
                else:
                    nc.vector.tensor_scalar(o_sb[:, ob, :], o_ps[ob][:],
                                            0.0, None, OP.add)
            nc.sync.dma_start(out.ap()[:, 0:NOB, :], o_sb[:, 0:NOB, :])
            nc.scalar.dma_start(out.ap()[:, NOB:2 * NOB, :],
                                o_sb[:, NOB:2 * NOB, :])

    nc.finalize()
    return nc


def _ptile(a2d, inner):
    """[R, C] row-major -> [P, R//P, C] partition-major, contiguous."""
    r, c = a2d.shape
    return np.ascontiguousarray(
        a2d.reshape(r // P, P, c).transpose(1, 0, 2)).astype(inner)


def _shard_inputs(inputs_q, inputs_kv, attention_mask, Wq, bq, Wk, bk, Wv, bv,
                  Wo, bo):
    bf16 = ml_dtypes.bfloat16
    f32 = np.float32

    xqT = [_ptile(inputs_q[b].T, bf16) for b in range(B)]         # [P,KC,TQ]
    xkvT = [_ptile(inputs_kv[b].T, bf16)                          # [P,NTC,KC,512]
            .reshape(P, KC, NTC, 512).transpose(0, 2, 1, 3).copy()
            for b in range(B)]
    maskT = [_ptile(attention_mask[b].T.astype(np.float32), bf16)  # [P,NTB,TQ]
             for b in range(B)]
    in_maps = []
    for c in range(NCORES):
        b, g = c // 2, c % 2  # pair = (2b, 2b+1)
        sl = slice(g * GD, (g + 1) * GD)
        in_maps.append({
            "xqT": xqT[b],
            "xkvT": xkvT[b],
            "maskT": maskT[b],
            "Wq": _ptile(np.ascontiguousarray(Wq[:, sl]), bf16),
            "Wk": _ptile(np.ascontiguousarray(Wk[:, sl]), bf16),
            "Wv": _ptile(np.ascontiguousarray(Wv[:, sl]), bf16),
            # own 4 heads' ROWS of Wo, full 1024 out-cols: [P, GH, D] bf16
            "Wo": _ptile(np.ascontiguousarray(Wo[sl, :]), bf16),
            "bq": np.ascontiguousarray(
                bq[sl].reshape(GH, P).T).astype(f32),
            "bk": np.ascontiguousarray(
                bk[sl].reshape(GH, P).T).astype(f32),
            "bv": np.ascontiguousarray(bv[sl]).astype(f32),
        })
    return in_maps


def kernel(_trace=False, **inputs):
    global _CACHED_NC
    from concourse import bass_utils

    arrs = {k: np.asarray(v) for k, v in inputs.items()}
    in_maps = _shard_inputs(**arrs)

    if _CACHED_NC is None:
        _CACHED_NC = _build_nc()

    res = bass_utils.run_bass_kernel_spmd(
        _CACHED_NC, in_maps, core_ids=list(range(NCORES)), trace=_trace)

    bo = arrs["bo"].astype(np.float32)
    full = np.empty((B, TQ, D), np.float32)
    for b in range(B):
        # sum the pair's 4-head partials on the host (the "all-reduce")
        acc = np.zeros((TQ, D), np.float32)
        for c in (2 * b, 2 * b + 1):
            o = res.results[c]["out"]  # [P, 8, TQ] bf16, o-col = ob*128+p
            acc += o.transpose(2, 1, 0).reshape(TQ, D).astype(np.float32)
        full[b] = acc + bo
    if _trace:
        return full, res
    return full


# revision 28
# speedup vs baseline: 1.2158x; 1.0018x over previous
"""Distributed attention kernel for one TRN2 chip (8 NeuronCores).

Problem: multi-head cross-attention
  B=4, TQ=512, TKV=4096, D=1024, H=8 heads (head_dim=128)

Sharding (data-parallel x tensor-parallel, per the hint):
  core c in 0..7 -> (batch b = c // 2, head-group g = c % 2)
  Each core computes heads [4g, 4g+4) for its batch (Wq/Wk/Wv column
  shards), pair-exchanges normalized U with core (b, 1-g) via AllGather,
  and computes its own 512-column slice of the output projection.

Pipeline structure (the key perf idea vs a phase-separated kernel):
  The attention math for T-chunk tc-1 is interleaved instruction-by-
  instruction with the K/V projection matmuls of T-chunk tc, so the
  ACT-engine exp and DVE mask/denominator work run entirely under the
  PE-bound projection stream.  The softmax denominator is accumulated
  on DVE (acc += p per double-step, bf16) instead of PE ones-matmuls,
  cutting ~27us of PE streaming.  PSUM budget is exactly 8 banks:
  4 U accumulators (whole kernel) + 2 S-tile banks (single-buffered,
  WAR hidden by the interleave) + 2 projection banks (double-buffered).

  Per-core PE work is the FLOP-minimal 13.96 GFLOP = ~178us of bf16
  streaming; everything else hides under it.

Tail: per-head finalize (den ones-matmul -> clamp -> fast-reciprocal ->
  gpsimd broadcast -> scale) pipelines into the attention drain of the
  last chunk.  There is NO on-device collective: each core emits the
  PARTIAL output projection over the full 1024 out-columns using only
  its own 4 heads (same FLOPs as a half-width 8-head projection), and
  the host sums the two partials of each pair during unsharding.  This
  removes the pair AllGather from the critical path entirely -- a
  tail-synchronizing collective costs its transfer time PLUS the full
  core-launch skew (measured 10-30us run-to-run), which no kernel-side
  scheduling can hide.
"""

import sys

if "/opt/trn_rl_repo" not in sys.path:
    sys.path.insert(0, "/opt/trn_rl_repo")

import numpy as np
import ml_dtypes
from contextlib import ExitStack

B, TQ, TKV, D, H = 4, 512, 4096, 1024, 8
HD = D // H            # 128 head dim
NCORES = 8
GH = H // 2            # heads per core = 4
GD = GH * HD           # 512 cols per head-group
P = 128
KC = D // P            # 8 contraction chunks
NTB = TKV // P         # 32 T-blocks
NTC = TKV // 512       # 8 T-chunks
NOB = GD // P          # 4 output blocks per core (own col half)
SCALE = float(1.0 / np.sqrt(HD))

_CACHED_NC = None


def _build_nc():
    from concourse import mybir, bacc
    from concourse.tile import TileContext

    bf = mybir.dt.bfloat16
    f32 = mybir.dt.float32
    AF = mybir.ActivationFunctionType
    OP = mybir.AluOpType

    nc = bacc.Bacc("TRN2", target_bir_lowering=False, debug=False,
                   num_devices=NCORES)

    # Host pre-tiles everything partition-major so DMAs are 128 x multi-KB
    # contiguous descriptors.
    xqT = nc.dram_tensor("xqT", [P, KC, TQ], bf, kind="ExternalInput")
    xkvT = nc.dram_tensor("xkvT", [P, NTC, KC, 512], bf, kind="ExternalInput")
    maskT = nc.dram_tensor("maskT", [P, NTB, TQ], bf, kind="ExternalInput")
    Wq = nc.dram_tensor("Wq", [P, KC, GD], bf, kind="ExternalInput")
    Wk = nc.dram_tensor("Wk", [P, KC, GD], bf, kind="ExternalInput")
    Wv = nc.dram_tensor("Wv", [P, KC, GD], bf, kind="ExternalInput")
    # own 4 heads' rows of Wo, FULL 1024 output columns
    Wo = nc.dram_tensor("Wo", [P, GH, D], bf, kind="ExternalInput")
    bq = nc.dram_tensor("bq", [P, GH], f32, kind="ExternalInput")
    bk = nc.dram_tensor("bk", [P, GH], f32, kind="ExternalInput")
    bv = nc.dram_tensor("bv", [GD], f32, kind="ExternalInput")
    out = nc.dram_tensor("out", [P, 2 * NOB, TQ], bf, kind="ExternalOutput")

    with TileContext(nc) as tc:
        with ExitStack() as ctx:
            persist = ctx.enter_context(tc.tile_pool(name="persist", bufs=1))
            kvchunk = ctx.enter_context(tc.tile_pool(name="kvchunk", bufs=2))
            work = ctx.enter_context(tc.tile_pool(name="work", bufs=3))
            # PSUM: exactly 8 banks.
            upool = ctx.enter_context(
                tc.tile_pool(name="upool", bufs=4, space="PSUM"))   # 4 banks
            spool = ctx.enter_context(
                tc.tile_pool(name="spool", bufs=1, space="PSUM"))   # 2 banks
            projp = ctx.enter_context(
                tc.tile_pool(name="projp", bufs=2, space="PSUM"))   # 2 banks
            dram = ctx.enter_context(
                tc.tile_pool(name="dram", bufs=1, space="DRAM"))

            # ---- startup DMAs on two HWDGE queues (sync + scalar) ------
            # Tiny bias DMAs go FIRST (a small transfer queued behind a 1MB
            # one completes ~10us late and its dependent copy stalls the
            # PE's psum-recycling chain).  Then kc-sliced wq/xq so the Q
            # projection starts ~9us in, then the 1MB tensors split across
            # the two queues in need-order.
            wq_sb = persist.tile([P, KC, GD], bf)
            xq_sb = persist.tile([P, KC, TQ], bf)
            wk_sb = persist.tile([P, KC, GD], bf)
            wv_sb = persist.tile([P, KC, GD], bf)
            mask_q = [persist.tile([P, 8, TQ], bf, name=f"mask{q}")
                      for q in range(4)]
            # Each HWDGE queue has 4 DMA rings; the first round of DMAs
            # completes ~4.5us after issue (cold engines) and round 2 waits
            # for ring credits.  Round 1 therefore carries exactly the data
            # the first ~20us of compute needs; biases ride round 2.
            kv_tiles = {}
            t0c = kvchunk.tile([P, KC, 512], bf, name="xkv_t", tag="xkv")
            kv_tiles[0] = t0c
            t1c = kvchunk.tile([P, KC, 512], bf, name="xkv_t", tag="xkv")
            kv_tiles[1] = t1c
            # round 1
            nc.sync.dma_start(wq_sb[:, 0:4, :], Wq.ap()[:, 0:4, :])
            nc.scalar.dma_start(xq_sb[:, 0:4, :], xqT.ap()[:, 0:4, :])
            nc.sync.dma_start(wq_sb[:, 4:8, :], Wq.ap()[:, 4:8, :])
            nc.scalar.dma_start(xq_sb[:, 4:8, :], xqT.ap()[:, 4:8, :])
            nc.sync.dma_start(t0c[:], xkvT.ap()[:, 0, :, :])
            nc.scalar.dma_start(wk_sb[:], Wk.ap())
            nc.sync.dma_start(wv_sb[:], Wv.ap())
            nc.scalar.dma_start(mask_q[0][:], maskT.ap()[:, 0:8, :])
            # round 2
            bq_sb = persist.tile([P, GH], f32)
            bk_sb = persist.tile([P, GH], f32)
            bv_row = persist.tile([1, GD], f32)
            nc.sync.dma_start(bq_sb[:], bq.ap())
            nc.sync.dma_start(bk_sb[:], bk.ap())
            nc.sync.dma_start(bv_row[:], bv.ap().unsqueeze(0))
            nc.sync.dma_start(t1c[:], xkvT.ap()[:, 1, :, :])
            bv_rep = persist.tile([P, GD], f32)
            nc.gpsimd.partition_broadcast(bv_rep[:], bv_row[:])

            ones_bf = persist.tile([P, 1], bf)
            nc.vector.memset(ones_bf[:], 1.0)

            # den accumulators (bf16; positive sums, relative errors wash)
            acc = [persist.tile([P, 2, TQ], bf, name=f"acc{h}") for h in range(GH)]
            for h in range(GH):
                nc.vector.memset(acc[h][:], 0.0)

            # ---- Q^T = Wq_g^T x_q^T (+bq), kc-major over 4 upool banks --
            qt_sb = persist.tile([P, GH, TQ], bf)
            q_ps = [upool.tile([P, TQ], f32, name="q_ps", tag="u")
                    for _ in range(GH)]
            for kc in range(KC):
                for db in range(GH):
                    nc.tensor.matmul(q_ps[db][:],
                                     wq_sb[:, kc, db * P:(db + 1) * P],
                                     xq_sb[:, kc, :],
                                     start=(kc == 0), stop=(kc == KC - 1))
            for db in range(GH):
                nc.scalar.activation(qt_sb[:, db, :], q_ps[db][:],
                                     AF.Identity, bias=bq_sb[:, db:db + 1])

            # ---- persistent SBUF for the streamed phase ----------------
            kt_sb = persist.tile([P, GH, TKV], bf)
            v_sb = persist.tile([P, NTB, GD], bf)
            wo_sb = persist.tile([P, GH, D], bf)
            ut_sb = persist.tile([P, GH, TQ], bf)
            o_sb = persist.tile([P, 2 * NOB, TQ], bf)

            u_ps = [None] * GH

            # attention double-step state machine (lag-2 U behind S)
            s_tiles = {}
            p_tiles = {}
            p7_tiles = {}

            def emit_S(ds):
                h, jp = divmod(ds, NTB // 2)
                j0 = 2 * jp
                t2 = spool.tile([P, 2, TQ], f32, name="s2_ps", tag="s")
                for k in range(2):
                    j = j0 + k
                    nc.tensor.matmul(t2[:, k, :],
                                     kt_sb[:, h, j * P:(j + 1) * P],
                                     qt_sb[:, h, :], start=True, stop=True)
                s_tiles[ds] = t2

            def emit_exp_mask(ds):
                h, jp = divmod(ds, NTB // 2)
                j0 = 2 * jp
                t2 = s_tiles.pop(ds)
                p_t = work.tile([P, 2, TQ], bf, tag="p_t", bufs=5)
                if jp >= 14:
                    # final-chunk drain: split the exp per psum bank so the
                    # next S matmul's WAR on the single S-slot releases
                    # after half an exp instead of a whole one
                    nc.scalar.activation(p_t[:, 0, :], t2[:, 0, :], AF.Exp,
                                         scale=SCALE)
                    nc.scalar.activation(p_t[:, 1, :], t2[:, 1, :], AF.Exp,
                                         scale=SCALE)
                else:
                    nc.scalar.activation(p_t[:], t2[:], AF.Exp, scale=SCALE)
                q, r0 = divmod(j0, 8)
                nc.vector.tensor_tensor(p_t[:], p_t[:],
                                        mask_q[q][:, r0:r0 + 2, :], OP.mult)
                if jp < 14:
                    nc.vector.tensor_tensor(acc[h][:], acc[h][:], p_t[:],
                                            OP.add)
                else:
                    # final chunk: the den contribution comes from direct
                    # ones-matmuls on the p tiles (PE is idle in the drain,
                    # DVE is the drain's scarce engine)
                    p7_tiles[(h, jp)] = p_t
                p_tiles[ds] = p_t

            def emit_U(ds):
                h, jp = divmod(ds, NTB // 2)
                j0 = 2 * jp
                if jp == 0:
                    u_ps[h] = upool.tile([P, TQ], f32, name="u_ps", tag="u")
                p_t = p_tiles.pop(ds)
                for k in range(2):
                    j = j0 + k
                    nc.tensor.matmul(u_ps[h][:],
                                     v_sb[:, j, h * P:(h + 1) * P],
                                     p_t[:, k, :],
                                     start=(j == 0), stop=(j == NTB - 1))

            # per-head finalize: den -> recip -> broadcast -> scale.
            # den = ones^T acc (chunks 0-6) + ones^T p (chunk 7's tiles,
            # whose DVE accumulation was skipped) in one psum group.
            def emit_fin(h):
                dps = projp.tile([P, TQ], f32, name="den_ps", tag="proj")
                for k in range(2):
                    nc.tensor.matmul(dps[0:1, :], ones_bf[:], acc[h][:, k, :],
                                     start=(k == 0), stop=False)
                for jp in (14, 15):
                    p7 = p7_tiles.pop((h, jp))
                    for k in range(2):
                        nc.tensor.matmul(dps[0:1, :], ones_bf[:], p7[:, k, :],
                                         start=False,
                                         stop=(jp == 15 and k == 1))
                den_cl = work.tile([1, TQ], f32, tag="den_cl", bufs=2)
                nc.vector.tensor_scalar(den_cl[:], dps[0:1, :], 1e-20, None,
                                        OP.max)
                recip = work.tile([1, TQ], f32, tag="recip", bufs=2)
                nc.vector.reciprocal_approx_fast(out=recip[:], in_=den_cl[:])
                rc = work.tile([P, TQ], f32, tag="rc_rep", bufs=2)
                nc.gpsimd.partition_broadcast(rc[:], recip[:])
                nc.vector.tensor_tensor(ut_sb[:, h, :], u_ps[h][:],
                                        rc[:], OP.mult)

            # ---- main streamed loop: proj(tc) interleaved with attn(tc-1)
            # per chunk: 8 proj groups (K db0-3, V tb0-3) and 8 double-steps
            # of the previous chunk's attention, round-robined so the PE
            # stream is dense and single-buffered S-psum never stalls.
            NDS = GH * NTB // 2   # 64 double-steps total
            # double-step visit order: chunk-major, head-minor
            ds_order = []
            for tcnk in range(NTC):
                for h in range(GH):
                    for pz in range(2):
                        ds_order.append(h * (NTB // 2) + tcnk * 2 + pz)
            s_q = list(ds_order)        # S-emission queue
            em_q = list(ds_order)       # exp/mask queue
            u_q = list(ds_order)        # U queue
            n_s = n_em = n_u = 0

            def pump(ns, nem, nu):
                # exp first: the next S matmul recycles the single-buffered
                # S psum slot, so its WAR must see the exp reader emitted.
                nonlocal n_s, n_em, n_u
                while n_em < nem and em_q:
                    emit_exp_mask(em_q.pop(0)); n_em += 1
                while n_s < ns and s_q:
                    emit_S(s_q.pop(0)); n_s += 1
                while n_u < nu and u_q:
                    emit_U(u_q.pop(0)); n_u += 1

            for tcnk in range(NTC):
                # stream next chunk + the mask quarter needed one chunk out
                if 1 <= tcnk < NTC - 1:
                    t = kvchunk.tile([P, KC, 512], bf, name="xkv_t", tag="xkv")
                    nc.sync.dma_start(t[:], xkvT.ap()[:, tcnk + 1, :, :])
                    kv_tiles[tcnk + 1] = t
                if tcnk in (1, 3, 5):
                    q = (tcnk + 1) // 2
                    nc.sync.dma_start(mask_q[q][:], maskT.ap()[:, 8 * q:8 * q + 8, :])
                if tcnk == 2:
                    nc.scalar.dma_start(wo_sb[:], Wo.ap())
                xkv_t = kv_tiles.pop(tcnk)

                # 8 proj groups interleaved with the attn pipeline of the
                # PREVIOUS chunk (its K/V tiles are fully in SBUF); U lags
                # the S matmuls by 2 double-steps so exp+mask always clear
                # the DVE/ACT queues before the PE reaches the U matmuls.
                base = (tcnk - 1) * 8
                for i in range(8):
                    # chunk 0's V groups borrow the 4 idle upool banks so
                    # the K and V psum-recycle chains start out independent
                    if tcnk == 0 and i >= 4:
                        ps = upool.tile([P, 512], f32, name="v0_ps", tag="u")
                    else:
                        ps = projp.tile([P, 512], f32, name="proj_ps", tag="proj")
                    if i < 4:
                        db = i
                        for kc in range(KC):
                            nc.tensor.matmul(ps[:],
                                             wk_sb[:, kc, db * P:(db + 1) * P],
                                             xkv_t[:, kc, :],
                                             start=(kc == 0), stop=(kc == KC - 1))
                        nc.scalar.activation(
                            kt_sb[:, db, tcnk * 512:(tcnk + 1) * 512], ps[:],
                            AF.Identity, bias=bk_sb[:, db:db + 1])
                    else:
                        tb = i - 4
                        for kc in range(KC):
                            nc.tensor.matmul(ps[:],
                                             xkv_t[:, kc, tb * P:(tb + 1) * P],
                                             wv_sb[:, kc, :],
                                             start=(kc == 0), stop=(kc == KC - 1))
                        nc.vector.tensor_tensor(
                            v_sb[:, tcnk * 4 + tb, :], ps[:], bv_rep[:], OP.add)
                    # pump the attention pipeline: one ds per group slot
                    tgt = base + i + 1
                    pump(tgt, tgt - 1, tgt - 2)

            # drain: remaining double-steps of chunk 7, finalizing each head
            # as soon as its last U accumulation is emitted (the finalize
            # chain fills the PE idle slots of the ACT-paced drain).
            fin_done = 0
            while s_q or em_q or u_q:
                pump(n_s + 1, n_em + 1, n_u + 1)
                # in ds_order, head h's last U is at position 56 + 2h + 1
                while fin_done < GH and n_u >= 58 + 2 * fin_done:
                    emit_fin(fin_done)
                    fin_done += 1
            while fin_done < GH:
                emit_fin(fin_done)
                fin_done += 1

            # ---- partial out-proj: own 4 heads x FULL 1024 out-cols ----
            # (no collective; the pair partner's 4-head partial is summed
            # on the host).  8 PSUM banks: 4 from upool (U slots freed by
            # the scales), 2 from projp, 2 from the spool tile's halves.
            o_ps = [upool.tile([P, TQ], f32, name="o_ps", tag="u")
                    for _ in range(NOB)]
            o_ps += [projp.tile([P, TQ], f32, name="o_ps2", tag="proj")
                     for _ in range(2)]
            o67 = spool.tile([P, 2, TQ], f32, name="o_ps3", tag="s")
            o_ps += [o67[:, 0, :], o67[:, 1, :]]
            for ob in range(2 * NOB):
                for lh in range(GH):
                    nc.tensor.matmul(o_ps[ob][:],
                                     wo_sb[:, lh, ob * P:(ob + 1) * P],
                                     ut_sb[:, lh, :],
                                     start=(lh == 0), stop=(lh == GH - 1))
                # alternate ACT/DVE so the 8 tail copies run ~2-wide
                if ob % 2 == 0:
                    nc.scalar.activation(o_sb[:, ob, :], o_ps[ob][:], AF.Copy)
                else:
                    nc.vector.tensor_scalar(o_sb[:, ob, :], o_ps[ob][:],
                                            0.0, None, OP.add)
                if ob % 2 == 1:
                    eng = nc.sync if (ob // 2) % 2 == 0 else nc.scalar
                    eng.dma_start(out.ap()[:, ob - 1:ob + 1, :],
                                  o_sb[:, ob - 1:ob + 1, :])

    nc.finalize()
    return nc


def _ptile(a2d, inner):
    """[R, C] row-major -> [P, R//P, C] partition-major, contiguous."""
    r, c = a2d.shape
    return np.ascontiguousarray(
        a2d.reshape(r // P, P, c).transpose(1, 0, 2)).astype(inner)


def _shard_inputs(inputs_q, inputs_kv, attention_mask, Wq, bq, Wk, bk, Wv, bv,
                  Wo, bo):
    bf16 = ml_dtypes.bfloat16
    f32 = np.float32

    xqT = [_ptile(inputs_q[b].T, bf16) for b in range(B)]         # [P,KC,TQ]
    xkvT = [_ptile(inputs_kv[b].T, bf16)                          # [P,NTC,KC,512]
            .reshape(P, KC, NTC, 512).transpose(0, 2, 1, 3).copy()
            for b in range(B)]
    maskT = [_ptile(attention_mask[b].T.astype(np.float32), bf16)  # [P,NTB,TQ]
             for b in range(B)]
    in_maps = []
    for c in range(NCORES):
        b, g = c // 2, c % 2  # pair = (2b, 2b+1)
        sl = slice(g * GD, (g + 1) * GD)
        in_maps.append({
            "xqT": xqT[b],
            "xkvT": xkvT[b],
            "maskT": maskT[b],
            "Wq": _ptile(np.ascontiguousarray(Wq[:, sl]), bf16),
            "Wk": _ptile(np.ascontiguousarray(Wk[:, sl]), bf16),
            "Wv": _ptile(np.ascontiguousarray(Wv[:, sl]), bf16),
            # own 4 heads' ROWS of Wo, full 1024 out-cols: [P, GH, D] bf16
            "Wo": _ptile(np.ascontiguousarray(Wo[sl, :]), bf16),
            "bq": np.ascontiguousarray(
                bq[sl].reshape(GH, P).T).astype(f32),
            "bk": np.ascontiguousarray(
                bk[sl].reshape(GH, P).T).astype(f32),
            "bv": np.ascontiguousarray(bv[sl]).astype(f32),
        })
    return in_maps


def kernel(_trace=False, **inputs):
    global _CACHED_NC
    from concourse import bass_utils

    arrs = {k: np.asarray(v) for k, v in inputs.items()}
    in_maps = _shard_inputs(**arrs)

    if _CACHED_NC is None:
        _CACHED_NC = _build_nc()

    res = bass_utils.run_bass_kernel_spmd(
        _CACHED_NC, in_maps, core_ids=list(range(NCORES)), trace=_trace)

    bo = arrs["bo"].astype(np.float32)
    full = np.empty((B, TQ, D), np.float32)
    for b in range(B):
        # sum the pair's 4-head partials on the host (the "all-reduce")
        acc = np.zeros((TQ, D), np.float32)
        for c in (2 * b, 2 * b + 1):
            o = res.results[c]["out"]  # [P, 8, TQ] bf16, o-col = ob*128+p
            acc += o.transpose(2, 1, 0).reshape(TQ, D).astype(np.float32)
        full[b] = acc + bo
    if _trace:
        return full, res
    return full


# revision 31
# speedup vs baseline: 1.2307x; 1.0122x over previous
"""Distributed attention kernel for one TRN2 chip (8 NeuronCores).

Problem: multi-head cross-attention
  B=4, TQ=512, TKV=4096, D=1024, H=8 heads (head_dim=128)

Sharding (data-parallel x tensor-parallel, per the hint):
  core c in 0..7 -> (batch b = c // 2, head-group g = c % 2)
  Each core computes heads [4g, 4g+4) for its batch (Wq/Wk/Wv column
  shards), pair-exchanges normalized U with core (b, 1-g) via AllGather,
  and computes its own 512-column slice of the output projection.

Pipeline structure (the key perf idea vs a phase-separated kernel):
  The attention math for T-chunk tc-1 is interleaved instruction-by-
  instruction with the K/V projection matmuls of T-chunk tc, so the
  ACT-engine exp and DVE mask/denominator work run entirely under the
  PE-bound projection stream.  The softmax denominator is accumulated
  on DVE (acc += p per double-step, bf16) instead of PE ones-matmuls,
  cutting ~27us of PE streaming.  PSUM budget is exactly 8 banks:
  4 U accumulators (whole kernel) + 2 S-tile banks (single-buffered,
  WAR hidden by the interleave) + 2 projection banks (double-buffered).

  Per-core PE work is the FLOP-minimal 13.96 GFLOP = ~178us of bf16
  streaming; everything else hides under it.

Tail: per-head finalize (den ones-matmul -> clamp -> fast-reciprocal ->
  gpsimd broadcast -> scale) pipelines into the attention drain of the
  last chunk.  There is NO on-device collective: each core emits the
  PARTIAL output projection over the full 1024 out-columns using only
  its own 4 heads (same FLOPs as a half-width 8-head projection), and
  the host sums the two partials of each pair during unsharding.  This
  removes the pair AllGather from the critical path entirely -- a
  tail-synchronizing collective costs its transfer time PLUS the full
  core-launch skew (measured 10-30us run-to-run), which no kernel-side
  scheduling can hide.
"""

import sys

if "/opt/trn_rl_repo" not in sys.path:
    sys.path.insert(0, "/opt/trn_rl_repo")

import numpy as np
import ml_dtypes
from contextlib import ExitStack

B, TQ, TKV, D, H = 4, 512, 4096, 1024, 8
HD = D // H            # 128 head dim
NCORES = 8
GH = H // 2            # heads per core = 4
GD = GH * HD           # 512 cols per head-group
P = 128
KC = D // P            # 8 contraction chunks
NTB = TKV // P         # 32 T-blocks
NTC = TKV // 512       # 8 T-chunks
NOB = GD // P          # 4 output blocks per core (own col half)
SCALE = float(1.0 / np.sqrt(HD))

_CACHED_NC = None


def _build_nc():
    from concourse import mybir, bacc
    from concourse.tile import TileContext

    bf = mybir.dt.bfloat16
    f32 = mybir.dt.float32
    AF = mybir.ActivationFunctionType
    OP = mybir.AluOpType

    nc = bacc.Bacc("TRN2", target_bir_lowering=False, debug=False,
                   num_devices=NCORES)

    # Host pre-tiles everything partition-major so DMAs are 128 x multi-KB
    # contiguous descriptors.
    xqT = nc.dram_tensor("xqT", [P, KC, TQ], bf, kind="ExternalInput")
    xkvT = nc.dram_tensor("xkvT", [P, NTC, KC, 512], bf, kind="ExternalInput")
    maskT = nc.dram_tensor("maskT", [P, NTB, TQ], bf, kind="ExternalInput")
    Wq = nc.dram_tensor("Wq", [P, KC, GD], bf, kind="ExternalInput")
    Wk = nc.dram_tensor("Wk", [P, KC, GD], bf, kind="ExternalInput")
    Wv = nc.dram_tensor("Wv", [P, KC, GD], bf, kind="ExternalInput")
    # own 4 heads' rows of Wo, FULL 1024 output columns
    Wo = nc.dram_tensor("Wo", [P, GH, D], bf, kind="ExternalInput")
    bqk = nc.dram_tensor("bqk", [P, 2, GH], f32, kind="ExternalInput")
    bv = nc.dram_tensor("bv", [GD], f32, kind="ExternalInput")
    out = nc.dram_tensor("out", [P, 2 * NOB, TQ], bf, kind="ExternalOutput")

    with TileContext(nc) as tc:
        with ExitStack() as ctx:
            persist = ctx.enter_context(tc.tile_pool(name="persist", bufs=1))
            kvchunk = ctx.enter_context(tc.tile_pool(name="kvchunk", bufs=2))
            work = ctx.enter_context(tc.tile_pool(name="work", bufs=3))
            # PSUM: exactly 8 banks.
            upool = ctx.enter_context(
                tc.tile_pool(name="upool", bufs=4, space="PSUM"))   # 4 banks
            spool = ctx.enter_context(
                tc.tile_pool(name="spool", bufs=1, space="PSUM"))   # 2 banks
            projp = ctx.enter_context(
                tc.tile_pool(name="projp", bufs=2, space="PSUM"))   # 2 banks
            dram = ctx.enter_context(
                tc.tile_pool(name="dram", bufs=1, space="DRAM"))

            # ---- startup DMAs on two HWDGE queues (sync + scalar) ------
            # Tiny bias DMAs go FIRST (a small transfer queued behind a 1MB
            # one completes ~10us late and its dependent copy stalls the
            # PE's psum-recycling chain).  Then kc-sliced wq/xq so the Q
            # projection starts ~9us in, then the 1MB tensors split across
            # the two queues in need-order.
            wq_sb = persist.tile([P, KC, GD], bf)
            xq_sb = persist.tile([P, KC, TQ], bf)
            wk_sb = persist.tile([P, KC, GD], bf)
            wv_sb = persist.tile([P, KC, GD], bf)
            mask_q = [persist.tile([P, 8, TQ], bf, name=f"mask{q}")
                      for q in range(4)]
            # Each HWDGE queue has 4 DMA rings; the first round of DMAs
            # completes ~4.5us after issue (cold engines) and round 2 waits
            # for ring credits.  Round 1 therefore carries exactly the data
            # the first ~20us of compute needs; biases ride round 2.
            kv_tiles = {}
            t0c = kvchunk.tile([P, KC, 512], bf, name="xkv_t", tag="xkv")
            kv_tiles[0] = t0c
            t1c = kvchunk.tile([P, KC, 512], bf, name="xkv_t", tag="xkv")
            kv_tiles[1] = t1c
            # round 1: the 3 critical 1MB tensors + the tiny packed biases
            # per queue (the small one completes with round-1 latency; if a
            # bias rode round 2 it would land ~25us and stall the Q-copy ->
            # upool-WAR chain)
            bqk_sb = persist.tile([P, 2, GH], f32)
            bv_row = persist.tile([1, GD], f32)
            nc.sync.dma_start(wq_sb[:, 0:4, :], Wq.ap()[:, 0:4, :])
            nc.scalar.dma_start(xq_sb[:, 0:4, :], xqT.ap()[:, 0:4, :])
            nc.sync.dma_start(wq_sb[:, 4:8, :], Wq.ap()[:, 4:8, :])
            nc.scalar.dma_start(xq_sb[:, 4:8, :], xqT.ap()[:, 4:8, :])
            nc.sync.dma_start(t0c[:], xkvT.ap()[:, 0, :, :])
            nc.scalar.dma_start(wk_sb[:], Wk.ap())
            nc.sync.dma_start(bqk_sb[:], bqk.ap())
            nc.scalar.dma_start(bv_row[:], bv.ap().unsqueeze(0))
            # round 2
            nc.sync.dma_start(wv_sb[:], Wv.ap())
            nc.scalar.dma_start(mask_q[0][:], maskT.ap()[:, 0:8, :])
            nc.sync.dma_start(t1c[:], xkvT.ap()[:, 1, :, :])
            bv_rep = persist.tile([P, GD], f32)
            nc.gpsimd.partition_broadcast(bv_rep[:], bv_row[:])
            bq_sb = bqk_sb[:, 0, :]
            bk_sb = bqk_sb[:, 1, :]

            ones_bf = persist.tile([P, 1], bf)
            nc.vector.memset(ones_bf[:], 1.0)

            # den accumulators (bf16; positive sums, relative errors wash)
            acc = [persist.tile([P, 2, TQ], bf, name=f"acc{h}") for h in range(GH)]
            for h in range(GH):
                nc.vector.memset(acc[h][:], 0.0)

            # ---- Q^T = Wq_g^T x_q^T (+bq), kc-major over 4 upool banks --
            qt_sb = persist.tile([P, GH, TQ], bf)
            q_ps = [upool.tile([P, TQ], f32, name="q_ps", tag="u")
                    for _ in range(GH)]
            for kc in range(KC):
                for db in range(GH):
                    nc.tensor.matmul(q_ps[db][:],
                                     wq_sb[:, kc, db * P:(db + 1) * P],
                                     xq_sb[:, kc, :],
                                     start=(kc == 0), stop=(kc == KC - 1))
            for db in range(GH):
                nc.scalar.activation(qt_sb[:, db, :], q_ps[db][:],
                                     AF.Identity, bias=bq_sb[:, db:db + 1])

            # ---- persistent SBUF for the streamed phase ----------------
            kt_sb = persist.tile([P, GH, TKV], bf)
            v_sb = persist.tile([P, NTB, GD], bf)
            wo_sb = persist.tile([P, GH, D], bf)
            ut_sb = persist.tile([P, GH, TQ], bf)
            o_sb = persist.tile([P, 2 * NOB, TQ], bf)

            u_ps = [None] * GH

            # attention double-step state machine (lag-2 U behind S)
            s_tiles = {}
            p_tiles = {}
            p7_tiles = {}

            def emit_S(ds):
                h, jp = divmod(ds, NTB // 2)
                j0 = 2 * jp
                t2 = spool.tile([P, 2, TQ], f32, name="s2_ps", tag="s")
                for k in range(2):
                    j = j0 + k
                    nc.tensor.matmul(t2[:, k, :],
                                     kt_sb[:, h, j * P:(j + 1) * P],
                                     qt_sb[:, h, :], start=True, stop=True)
                s_tiles[ds] = t2

            def emit_exp_mask(ds):
                h, jp = divmod(ds, NTB // 2)
                j0 = 2 * jp
                t2 = s_tiles.pop(ds)
                p_t = work.tile([P, 2, TQ], bf, tag="p_t", bufs=5)
                if jp >= 14:
                    # final-chunk drain: split the exp per psum bank so the
                    # next S matmul's WAR on the single S-slot releases
                    # after half an exp instead of a whole one
                    nc.scalar.activation(p_t[:, 0, :], t2[:, 0, :], AF.Exp,
                                         scale=SCALE)
                    nc.scalar.activation(p_t[:, 1, :], t2[:, 1, :], AF.Exp,
                                         scale=SCALE)
                else:
                    nc.scalar.activation(p_t[:], t2[:], AF.Exp, scale=SCALE)
                q, r0 = divmod(j0, 8)
                nc.vector.tensor_tensor(p_t[:], p_t[:],
                                        mask_q[q][:, r0:r0 + 2, :], OP.mult)
                if jp < 14:
                    nc.vector.tensor_tensor(acc[h][:], acc[h][:], p_t[:],
                                            OP.add)
                else:
                    # final chunk: the den contribution comes from direct
                    # ones-matmuls on the p tiles (PE is idle in the drain,
                    # DVE is the drain's scarce engine)
                    p7_tiles[(h, jp)] = p_t
                p_tiles[ds] = p_t

            def emit_U(ds):
                h, jp = divmod(ds, NTB // 2)
                j0 = 2 * jp
                if jp == 0:
                    u_ps[h] = upool.tile([P, TQ], f32, name="u_ps", tag="u")
                p_t = p_tiles.pop(ds)
                for k in range(2):
                    j = j0 + k
                    nc.tensor.matmul(u_ps[h][:],
                                     v_sb[:, j, h * P:(h + 1) * P],
                                     p_t[:, k, :],
                                     start=(j == 0), stop=(j == NTB - 1))

            # per-head finalize: den -> recip -> broadcast -> scale.
            # den = ones^T acc (chunks 0-6) + ones^T p (chunk 7's tiles,
            # whose DVE accumulation was skipped) in one psum group.
            def emit_fin(h):
                dps = projp.tile([P, TQ], f32, name="den_ps", tag="proj")
                for k in range(2):
                    nc.tensor.matmul(dps[0:1, :], ones_bf[:], acc[h][:, k, :],
                                     start=(k == 0), stop=False)
                for jp in (14, 15):
                    p7 = p7_tiles.pop((h, jp))
                    for k in range(2):
                        nc.tensor.matmul(dps[0:1, :], ones_bf[:], p7[:, k, :],
                                         start=False,
                                         stop=(jp == 15 and k == 1))
                den_cl = work.tile([1, TQ], f32, tag="den_cl", bufs=2)
                nc.vector.tensor_scalar(den_cl[:], dps[0:1, :], 1e-20, None,
                                        OP.max)
                recip = work.tile([1, TQ], f32, tag="recip", bufs=2)
                nc.vector.reciprocal_approx_fast(out=recip[:], in_=den_cl[:])
                rc = work.tile([P, TQ], f32, tag="rc_rep", bufs=2)
                nc.gpsimd.partition_broadcast(rc[:], recip[:])
                nc.vector.tensor_tensor(ut_sb[:, h, :], u_ps[h][:],
                                        rc[:], OP.mult)

            # ---- main streamed loop: proj(tc) interleaved with attn(tc-1)
            # per chunk: 8 proj groups (K db0-3, V tb0-3) and 8 double-steps
            # of the previous chunk's attention, round-robined so the PE
            # stream is dense and single-buffered S-psum never stalls.
            NDS = GH * NTB // 2   # 64 double-steps total
            # double-step visit order: chunk-major, head-minor
            ds_order = []
            for tcnk in range(NTC):
                for h in range(GH):
                    for pz in range(2):
                        ds_order.append(h * (NTB // 2) + tcnk * 2 + pz)
            s_q = list(ds_order)        # S-emission queue
            em_q = list(ds_order)       # exp/mask queue
            u_q = list(ds_order)        # U queue
            n_s = n_em = n_u = 0

            def pump(ns, nem, nu):
                # exp first: the next S matmul recycles the single-buffered
                # S psum slot, so its WAR must see the exp reader emitted.
                nonlocal n_s, n_em, n_u
                while n_em < nem and em_q:
                    emit_exp_mask(em_q.pop(0)); n_em += 1
                while n_s < ns and s_q:
                    emit_S(s_q.pop(0)); n_s += 1
                while n_u < nu and u_q:
                    emit_U(u_q.pop(0)); n_u += 1

            for tcnk in range(NTC):
                # stream next chunk + the mask quarter needed one chunk out
                if 1 <= tcnk < NTC - 1:
                    t = kvchunk.tile([P, KC, 512], bf, name="xkv_t", tag="xkv")
                    nc.sync.dma_start(t[:], xkvT.ap()[:, tcnk + 1, :, :])
                    kv_tiles[tcnk + 1] = t
                if tcnk in (1, 3, 5):
                    q = (tcnk + 1) // 2
                    nc.sync.dma_start(mask_q[q][:], maskT.ap()[:, 8 * q:8 * q + 8, :])
                if tcnk == 2:
                    nc.scalar.dma_start(wo_sb[:], Wo.ap())
                xkv_t = kv_tiles.pop(tcnk)

                # 8 proj groups interleaved with the attn pipeline of the
                # PREVIOUS chunk (its K/V tiles are fully in SBUF); U lags
                # the S matmuls by 2 double-steps so exp+mask always clear
                # the DVE/ACT queues before the PE reaches the U matmuls.
                base = (tcnk - 1) * 8
                for i in range(8):
                    # chunk 0's V groups borrow the 4 idle upool banks so
                    # the K and V psum-recycle chains start out independent
                    if tcnk == 0 and i >= 4:
                        ps = upool.tile([P, 512], f32, name="v0_ps", tag="u")
                    else:
                        ps = projp.tile([P, 512], f32, name="proj_ps", tag="proj")
                    if i < 4:
                        db = i
                        for kc in range(KC):
                            nc.tensor.matmul(ps[:],
                                             wk_sb[:, kc, db * P:(db + 1) * P],
                                             xkv_t[:, kc, :],
                                             start=(kc == 0), stop=(kc == KC - 1))
                        nc.scalar.activation(
                            kt_sb[:, db, tcnk * 512:(tcnk + 1) * 512], ps[:],
                            AF.Identity, bias=bk_sb[:, db:db + 1])
                    else:
                        tb = i - 4
                        for kc in range(KC):
                            nc.tensor.matmul(ps[:],
                                             xkv_t[:, kc, tb * P:(tb + 1) * P],
                                             wv_sb[:, kc, :],
                                             start=(kc == 0), stop=(kc == KC - 1))
                        nc.vector.tensor_tensor(
                            v_sb[:, tcnk * 4 + tb, :], ps[:], bv_rep[:], OP.add)
                    # pump the attention pipeline: one ds per group slot
                    tgt = base + i + 1
                    pump(tgt, tgt - 1, tgt - 2)

            # drain: remaining double-steps of chunk 7, finalizing each head
            # as soon as its last U accumulation is emitted (the finalize
            # chain fills the PE idle slots of the ACT-paced drain).
            fin_done = 0
            while s_q or em_q or u_q:
                pump(n_s + 1, n_em + 1, n_u + 1)
                # in ds_order, head h's last U is at position 56 + 2h + 1
                while fin_done < GH and n_u >= 58 + 2 * fin_done:
                    emit_fin(fin_done)
                    fin_done += 1
            while fin_done < GH:
                emit_fin(fin_done)
                fin_done += 1

            # ---- partial out-proj: own 4 heads x FULL 1024 out-cols ----
            # (no collective; the pair partner's 4-head partial is summed
            # on the host).  8 PSUM banks: 4 from upool (U slots freed by
            # the scales), 2 from projp, 2 from the spool tile's halves.
            o_ps = [upool.tile([P, TQ], f32, name="o_ps", tag="u")
                    for _ in range(NOB)]
            o_ps += [projp.tile([P, TQ], f32, name="o_ps2", tag="proj")
                     for _ in range(2)]
            o67 = spool.tile([P, 2, TQ], f32, name="o_ps3", tag="s")
            o_ps += [o67[:, 0, :], o67[:, 1, :]]
            for ob in range(2 * NOB):
                for lh in range(GH):
                    nc.tensor.matmul(o_ps[ob][:],
                                     wo_sb[:, lh, ob * P:(ob + 1) * P],
                                     ut_sb[:, lh, :],
                                     start=(lh == 0), stop=(lh == GH - 1))
                # alternate ACT/DVE so the 8 tail copies run ~2-wide
                if ob % 2 == 0:
                    nc.scalar.activation(o_sb[:, ob, :], o_ps[ob][:], AF.Copy)
                else:
                    nc.vector.tensor_scalar(o_sb[:, ob, :], o_ps[ob][:],
                                            0.0, None, OP.add)
                if ob % 2 == 1:
                    eng = nc.sync if (ob // 2) % 2 == 0 else nc.scalar
                    eng.dma_start(out.ap()[:, ob - 1:ob + 1, :],
                                  o_sb[:, ob - 1:ob + 1, :])

    nc.finalize()
    return nc


def _ptile(a2d, inner):
    """[R, C] row-major -> [P, R//P, C] partition-major, contiguous."""
    r, c = a2d.shape
    return np.ascontiguousarray(
        a2d.reshape(r // P, P, c).transpose(1, 0, 2)).astype(inner)


def _shard_inputs(inputs_q, inputs_kv, attention_mask, Wq, bq, Wk, bk, Wv, bv,
                  Wo, bo):
    bf16 = ml_dtypes.bfloat16
    f32 = np.float32

    xqT = [_ptile(inputs_q[b].T, bf16) for b in range(B)]         # [P,KC,TQ]
    xkvT = [_ptile(inputs_kv[b].T, bf16)                          # [P,NTC,KC,512]
            .reshape(P, KC, NTC, 512).transpose(0, 2, 1, 3).copy()
            for b in range(B)]
    maskT = [_ptile(attention_mask[b].T.astype(np.float32), bf16)  # [P,NTB,TQ]
             for b in range(B)]
    in_maps = []
    for c in range(NCORES):
        b, g = c // 2, c % 2  # pair = (2b, 2b+1)
        sl = slice(g * GD, (g + 1) * GD)
        in_maps.append({
            "xqT": xqT[b],
            "xkvT": xkvT[b],
            "maskT": maskT[b],
            "Wq": _ptile(np.ascontiguousarray(Wq[:, sl]), bf16),
            "Wk": _ptile(np.ascontiguousarray(Wk[:, sl]), bf16),
            "Wv": _ptile(np.ascontiguousarray(Wv[:, sl]), bf16),
            # own 4 heads' ROWS of Wo, full 1024 out-cols: [P, GH, D] bf16
            "Wo": _ptile(np.ascontiguousarray(Wo[sl, :]), bf16),
            "bqk": np.ascontiguousarray(np.stack(
                [bq[sl].reshape(GH, P).T,
                 bk[sl].reshape(GH, P).T], axis=1)).astype(f32),
            "bv": np.ascontiguousarray(bv[sl]).astype(f32),
        })
    return in_maps


def kernel(_trace=False, **inputs):
    global _CACHED_NC
    from concourse import bass_utils

    arrs = {k: np.asarray(v) for k, v in inputs.items()}
    in_maps = _shard_inputs(**arrs)

    if _CACHED_NC is None:
        _CACHED_NC = _build_nc()

    res = bass_utils.run_bass_kernel_spmd(
        _CACHED_NC, in_maps, core_ids=list(range(NCORES)), trace=_trace)

    bo = arrs["bo"].astype(np.float32)
    full = np.empty((B, TQ, D), np.float32)
    for b in range(B):
        # sum the pair's 4-head partials on the host (the "all-reduce")
        acc = np.zeros((TQ, D), np.float32)
        for c in (2 * b, 2 * b + 1):
            o = res.results[c]["out"]  # [P, 8, TQ] bf16, o-col = ob*128+p
            acc += o.transpose(2, 1, 0).reshape(TQ, D).astype(np.float32)
        full[b] = acc + bo
    if _trace:
        return full, res
    return full
